# revision 2
# baseline (speedup 1.0000x reference)
"""Trainium2 Bass kernel for windowed local self-attention MLP.

Reference computation (per batch b, S=2048 tokens, D=H=256, A=16, W=33):
    h   = relu(x @ W1 + b1)
    Q   = h @ Wq ; Khat = h @ Wk ; Vhat = h @ Wv        (windowed K/V are
          shifted views of Khat/Vhat -- the algebraic collapse of the
          reference's [B,S,W,H] window tensor)
    logit[s,m] = Q[s].Khat[s+A-m]/sqrt(H)  (zero outside [0,S), m=0..32)
    attn = softmax(logit) ; att[s] = sum_m attn[s,m] Vhat[s+A-m]
    out = relu(att @ Wh + bh) @ Wo + bo

Sharding: data-parallel over batch, one batch element per NeuronCore (B=8,
8 cores), weights replicated, no collectives.

Layout strategy: activations live feature-on-partition ("transposed",
[256=2x128, S]) so every dense matmul contracts over partitions. x is
transposed host-side. The band attention runs per 128-token chunk with a
256-wide token window aligned to shifted 128-tiles (shift -A), an additive
-1e9 band mask, exp with fused row-sum (accum_out), and a PE transpose of the
normalized weights feeding bf16 [V-tile]^T @ [e]^T matmuls.

Dense matmuls use float32r (full PE rate, ~2e-4 rel err); the attention
apply uses bf16. Final projection is emitted transposed [2, S] and
un-transposed host-side; bo is added host-side.
"""
import sys

if "/opt/trn_rl_repo" not in sys.path:
    sys.path.insert(0, "/opt/trn_rl_repo")

import numpy as np

import concourse.bass as bass  # noqa: F401  (engine types referenced via nc)
import concourse.mybir as mybir
import concourse.tile as tile
from concourse import bacc
from concourse.bass_utils import run_bass_kernel_spmd
from concourse.masks import make_identity

P = 128
S = 2048  # tokens per core
D = 256  # model dim
A = 16  # half window
NC = 16  # token chunks per core
NCORES = 8

PADW = P * (NC + 1)  # 2176: padded token axis, col = token + A
F32 = mybir.dt.float32
F32R = mybir.dt.float32r
BF16 = mybir.dt.bfloat16

_CACHED_NC = None


def _build_nc():
    nc = bacc.Bacc(
        "TRN2",
        target_bir_lowering=False,
        debug=False,
        enable_asserts=False,
        num_devices=NCORES,
    )
    xt = nc.dram_tensor("xt", [D, S], F32, kind="ExternalInput").ap()
    w1 = nc.dram_tensor("w1", [D, D], F32, kind="ExternalInput").ap()
    wq = nc.dram_tensor("wq", [D, D], F32, kind="ExternalInput").ap()
    wk = nc.dram_tensor("wk", [D, D], F32, kind="ExternalInput").ap()
    wv = nc.dram_tensor("wv", [D, D], F32, kind="ExternalInput").ap()
    wh = nc.dram_tensor("wh", [D, D], F32, kind="ExternalInput").ap()
    wo = nc.dram_tensor("wo", [D, 2], F32, kind="ExternalInput").ap()
    b1 = nc.dram_tensor("b1", [D], F32, kind="ExternalInput").ap()
    bh = nc.dram_tensor("bh", [D], F32, kind="ExternalInput").ap()
    zer = nc.dram_tensor("zer", [P, 2 * P], F32, kind="ExternalInput").ap()
    out_t = nc.dram_tensor("out_t", [2, S], F32, kind="ExternalOutput").ap()

    with tile.TileContext(nc) as tc:
        with (
            tc.tile_pool(name="persist", bufs=1) as persist,
            tc.tile_pool(name="work", bufs=4) as work,
            tc.tile_pool(name="psum", bufs=8, space="PSUM") as psum,
        ):
            # ---------------- constants / weights ----------------
            w1_sb = persist.tile([P, 2, D], F32R)
            wq_sb = persist.tile([P, 2, D], F32R)
            wk_sb = persist.tile([P, 2, D], F32R)
            wv_sb = persist.tile([P, 2, D], F32R)
            wh_sb = persist.tile([P, 2, D], F32R)
            wo_sb = persist.tile([P, 2, 2], F32R)
            for wsb, wdr in ((w1_sb, w1), (wq_sb, wq), (wk_sb, wk),
                             (wv_sb, wv), (wh_sb, wh), (wo_sb, wo)):
                nc.sync.dma_start(
                    wsb[:], wdr.rearrange("(k p) h -> p k h", p=P).bitcast(F32R)
                )
            b1_sb = persist.tile([P, 2], F32)
            bh_sb = persist.tile([P, 2], F32)
            nc.sync.dma_start(b1_sb[:], b1.rearrange("(hm p) -> p hm", p=P))
            nc.sync.dma_start(bh_sb[:], bh.rearrange("(hm p) -> p hm", p=P))

            ident_bf = persist.tile([P, P], BF16)
            make_identity(nc, ident_bf[:])

            # additive band mask [P, 2*P]: 0 where p <= j <= p + 2A else -1e9
            maskb = persist.tile([P, 2 * P], F32)
            nc.gpsimd.memset(maskb[:], 0.0)
            nc.gpsimd.affine_select(
                out=maskb[:], in_=maskb[:], compare_op=mybir.AluOpType.is_ge,
                fill=-1e9, base=0, pattern=[[1, 2 * P]], channel_multiplier=-1,
            )
            nc.gpsimd.affine_select(
                out=maskb[:], in_=maskb[:], compare_op=mybir.AluOpType.is_ge,
                fill=-1e9, base=2 * A, pattern=[[-1, 2 * P]], channel_multiplier=1,
            )

            # ---------------- persistent activations ----------------
            xt_sb = persist.tile([P, 2, S], F32R)
            ht = persist.tile([P, 2, PADW], F32R)  # col = token + A
            qt = persist.tile([P, 2, S], F32R)
            kt = persist.tile([P, 2, PADW], F32R)  # col = token + A
            vs = persist.tile([P, NC + 1, D], BF16)  # tile t row p = token t*128+p-A
            att = persist.tile([P, 2, S], F32R)
            hid = persist.tile([P, 2, S], F32R)
            ot_sb = persist.tile([2, S], F32)

            # zero the pad regions of ht/kt (cols [0,A) and [S+A, PADW))
            for ko in range(2):
                nc.sync.dma_start(ht[:, ko, 0:A], zer[:, 0:A].bitcast(F32R))
                nc.sync.dma_start(kt[:, ko, 0:A], zer[:, 0:A].bitcast(F32R))
                nc.sync.dma_start(
                    ht[:, ko, S + A:PADW], zer[:, 0:PADW - S - A].bitcast(F32R)
                )
                nc.sync.dma_start(
                    kt[:, ko, S + A:PADW], zer[:, 0:PADW - S - A].bitcast(F32R)
                )

            # x load, split for pipelining
            for t in range(4):
                nc.sync.dma_start(
                    xt_sb[:, :, t * 512:(t + 1) * 512],
                    xt.rearrange("(ko p) s -> p ko s", p=P)[
                        :, :, t * 512:(t + 1) * 512
                    ].bitcast(F32R),
                )

            # ---------------- P1: ht = relu(W1^T @ xt + b1) ----------------
            for hm in range(2):
                for t in range(4):
                    ps = psum.tile([P, 512], F32, tag="bank")
                    for k in range(2):
                        nc.tensor.matmul(
                            ps[:], w1_sb[:, k, hm * P:(hm + 1) * P],
                            xt_sb[:, k, t * 512:(t + 1) * 512],
                            start=(k == 0), stop=(k == 1),
                        )
                    nc.scalar.activation(
                        ht[:, hm, A + t * 512:A + (t + 1) * 512], ps[:],
                        mybir.ActivationFunctionType.Relu, bias=b1_sb[:, hm:hm + 1],
                    )

            # ---------------- P2/P3: qt (ACT copy), kt (DVE copy) ----------------
            for hm in range(2):
                for t in range(4):
                    psq = psum.tile([P, 512], F32, tag="bank")
                    for k in range(2):
                        nc.tensor.matmul(
                            psq[:], wq_sb[:, k, hm * P:(hm + 1) * P],
                            ht[:, k, A + t * 512:A + (t + 1) * 512],
                            start=(k == 0), stop=(k == 1),
                        )
                    nc.scalar.copy(qt[:, hm, t * 512:(t + 1) * 512], psq[:])

                    psk = psum.tile([P, 512], F32, tag="bank")
                    for k in range(2):
                        nc.tensor.matmul(
                            psk[:], wk_sb[:, k, hm * P:(hm + 1) * P],
                            ht[:, k, A + t * 512:A + (t + 1) * 512],
                            start=(k == 0), stop=(k == 1),
                        )
                    nc.vector.tensor_copy(
                        kt[:, hm, A + t * 512:A + (t + 1) * 512], psk[:]
                    )

            # ---------------- P4: shifted V tiles (natural layout, bf16) ----------
            for t in range(NC + 1):
                psv = psum.tile([P, D], F32, tag="bank")
                for k in range(2):
                    nc.tensor.matmul(
                        psv[:], ht[:, k, t * P:(t + 1) * P], wv_sb[:, k, :],
                        start=(k == 0), stop=(k == 1),
                    )
                nc.vector.tensor_copy(vs[:, t, :], psv[:])

            # ---------------- P5: band attention, per 128-token chunk ------------
            for g in range(4):  # groups of 4 chunks share att psum banks
                psa = [psum.tile([P, 512], F32, tag="bank", name=f"att{g}_{fm}")
                       for fm in range(2)]
                for ci in range(4):
                    c = 4 * g + ci
                    # logits for chunk c: tokens c*128 + [0,128), window cols
                    # kt[c*128 : c*128+256) == tokens c*128-A .. c*128+240-A
                    psl = psum.tile([P, 2 * P], F32, tag="bank", name="logit")
                    for k in range(2):
                        nc.tensor.matmul(
                            psl[:], qt[:, k, c * P:(c + 1) * P],
                            kt[:, k, c * P:c * P + 2 * P],
                            start=(k == 0), stop=(k == 1),
                        )
                    pre = work.tile([P, 2 * P], F32, tag="pre")
                    nc.vector.tensor_add(pre[:], psl[:], maskb[:])
                    e = work.tile([P, 2 * P], F32, tag="e")
                    den = work.tile([P, 1], F32, tag="den")
                    nc.scalar.activation(
                        e[:], pre[:], mybir.ActivationFunctionType.Exp,
                        scale=0.0625, accum_out=den[:],
                    )
                    rec = work.tile([P, 1], F32, tag="rec")
                    nc.vector.reciprocal(rec[:], den[:])
                    enb = work.tile([P, 2 * P], BF16, tag="enb")
                    nc.vector.tensor_scalar_mul(enb[:], e[:], rec[:])
                    # transpose the two 128-blocks of normalized weights
                    pse = psum.tile([P, 2, P], BF16, tag="bank", name="etr")
                    for b in range(2):
                        nc.tensor.transpose(
                            pse[:, b, :], enb[:, b * P:(b + 1) * P], ident_bf[:]
                        )
                    et = work.tile([P, 2, P], BF16, tag="et")
                    nc.vector.tensor_copy(et[:], pse[:])
                    # att^T[fm] += Vtile[c+b]^T @ et[b]
                    for fm in range(2):
                        for b in range(2):
                            nc.tensor.matmul(
                                psa[fm][:, ci * P:(ci + 1) * P],
                                vs[:, c + b, fm * P:(fm + 1) * P],
                                et[:, b, :],
                                start=(b == 0), stop=(b == 1),
                            )
                for fm in range(2):
                    nc.vector.tensor_copy(
                        att[:, fm, g * 512:(g + 1) * 512], psa[fm][:]
                    )

            # ---------------- P6: hid = relu(Wh^T @ att + bh) ----------------
            for hm in range(2):
                for t in range(4):
                    ps = psum.tile([P, 512], F32, tag="bank")
                    for k in range(2):
                        nc.tensor.matmul(
                            ps[:], wh_sb[:, k, hm * P:(hm + 1) * P],
                            att[:, k, t * 512:(t + 1) * 512],
                            start=(k == 0), stop=(k == 1),
                        )
                    nc.scalar.activation(
                        hid[:, hm, t * 512:(t + 1) * 512], ps[:],
                        mybir.ActivationFunctionType.Relu, bias=bh_sb[:, hm:hm + 1],
                    )

            # ---------------- P7: out^T = Wo^T @ hid ----------------
            for t in range(4):
                pso = psum.tile([2, 512], F32, tag="bank", name="outp")
                for k in range(2):
                    nc.tensor.matmul(
                        pso[:], wo_sb[:, k, :],
                        hid[:, k, t * 512:(t + 1) * 512],
                        start=(k == 0), stop=(k == 1),
                    )
                nc.scalar.copy(ot_sb[:, t * 512:(t + 1) * 512], pso[:])
            nc.sync.dma_start(out_t, ot_sb[:])

    nc.compile()
    return nc


def _get_nc():
    global _CACHED_NC
    if _CACHED_NC is None:
        _CACHED_NC = _build_nc()
    return _CACHED_NC


def kernel(x, W1, b1, Wq, Wk, Wv, Wh, bh, Wo, bo, **_unused):
    x = np.asarray(x, dtype=np.float32)
    W1 = np.asarray(W1, dtype=np.float32)
    Wq = np.asarray(Wq, dtype=np.float32)
    Wk = np.asarray(Wk, dtype=np.float32)
    Wv = np.asarray(Wv, dtype=np.float32)
    Wh = np.asarray(Wh, dtype=np.float32)
    Wo = np.asarray(Wo, dtype=np.float32)
    b1f = np.asarray(b1, dtype=np.float32).reshape(D)
    bhf = np.asarray(bh, dtype=np.float32).reshape(D)
    bof = np.asarray(bo, dtype=np.float32).reshape(2)
    zer = np.zeros((P, 2 * P), dtype=np.float32)

    nc = _get_nc()
    in_maps = []
    for b in range(NCORES):
        in_maps.append({
            "xt": np.ascontiguousarray(x[b].T),
            "w1": W1, "wq": Wq, "wk": Wk, "wv": Wv, "wh": Wh, "wo": Wo,
            "b1": b1f, "bh": bhf, "zer": zer,
        })
    res = run_bass_kernel_spmd(nc, in_maps, core_ids=list(range(NCORES)))
    global _LAST_RESULTS
    _LAST_RESULTS = res
    out = np.stack(
        [res.results[b]["out_t"].T + bof[None, :] for b in range(NCORES)], axis=0
    )
    return out.astype(np.float32)


if __name__ == "__main__":
    rng = np.random.default_rng(0)
    ins = {
        "x": rng.standard_normal((8, S, D), dtype=np.float32),
        "W1": (rng.standard_normal((D, D), dtype=np.float32) / 16),
        "b1": np.zeros((1, 1, D), np.float32),
        "Wq": (rng.standard_normal((D, D), dtype=np.float32) / 16),
        "Wk": (rng.standard_normal((D, D), dtype=np.float32) / 16),
        "Wv": (rng.standard_normal((D, D), dtype=np.float32) / 16),
        "Wh": (rng.standard_normal((D, D), dtype=np.float32) / 16),
        "bh": np.zeros((1, 1, D), np.float32),
        "Wo": (rng.standard_normal((D, 2), dtype=np.float32) / 16),
        "bo": np.zeros((1, 1, 2), np.float32),
    }
    y = kernel(**ins)
    print("kernel output", y.shape, y.dtype, float(np.abs(y).max()))


# revision 3
# speedup vs baseline: 1.0701x; 1.0701x over previous
"""Trainium2 Bass kernel for windowed local self-attention MLP.

Reference computation (per batch b, S=2048 tokens, D=H=256, A=16, W=33):
    h   = relu(x @ W1 + b1)
    Q   = h @ Wq ; Khat = h @ Wk ; Vhat = h @ Wv        (windowed K/V are
          shifted views of Khat/Vhat -- the algebraic collapse of the
          reference's [B,S,W,H] window tensor)
    logit[s,m] = Q[s].Khat[s+A-m]/sqrt(H)  (zero outside [0,S), m=0..32)
    attn = softmax(logit) ; att[s] = sum_m attn[s,m] Vhat[s+A-m]
    out = relu(att @ Wh + bh) @ Wo + bo

Sharding: data-parallel over batch, one batch element per NeuronCore (B=8,
8 cores), weights replicated, no collectives.

Layout: activations feature-on-partition ([256=2x128, S]) so dense matmuls
contract over partitions; x is transposed host-side. Band attention runs per
128-token chunk over a 256-token window aligned to shifted (-A) 128-tiles.
The additive -1e9 band mask is pre-loaded into PSUM by an identity matmul,
the QK logits accumulate on top, exp runs with a fused row-sum (accum_out),
and a PE transpose of the normalized fp16 weights feeds [V-tile]^T @ [e]^T
fp16 matmuls. Dense matmuls use float32r (full PE rate, ~2e-4 rel err).
Emission is a token-stripe wave so DMA/PE/ACT/DVE overlap across phases.
Final projection is emitted transposed [2, S]; un-transposed and bo added
host-side.
"""
import sys

if "/opt/trn_rl_repo" not in sys.path:
    sys.path.insert(0, "/opt/trn_rl_repo")

import numpy as np

import concourse.mybir as mybir
import concourse.tile as tile
from concourse import bacc
from concourse.bass_utils import run_bass_kernel_spmd

P = 128
S = 2048  # tokens per core
D = 256  # model dim
A = 16  # half window
NC = 16  # token chunks per core
NCORES = 8

PADW = P * (NC + 1)  # 2176: padded token axis, col = token + A
F32 = mybir.dt.float32
F32R = mybir.dt.float32r
FP16 = mybir.dt.float16

_CACHED_NC = None
_LAST_RESULTS = None


def _build_nc():
    nc = bacc.Bacc(
        "TRN2",
        target_bir_lowering=False,
        debug=False,
        enable_asserts=False,
        num_devices=NCORES,
    )
    xt = nc.dram_tensor("xt", [D, S], F32, kind="ExternalInput").ap()
    w1 = nc.dram_tensor("w1", [D, D], F32, kind="ExternalInput").ap()
    wq = nc.dram_tensor("wq", [D, D], F32, kind="ExternalInput").ap()
    wk = nc.dram_tensor("wk", [D, D], F32, kind="ExternalInput").ap()
    wv = nc.dram_tensor("wv", [D, D], F32, kind="ExternalInput").ap()
    wh = nc.dram_tensor("wh", [D, D], F32, kind="ExternalInput").ap()
    wo = nc.dram_tensor("wo", [D, 2], F32, kind="ExternalInput").ap()
    b1 = nc.dram_tensor("b1", [D], F32, kind="ExternalInput").ap()
    bh = nc.dram_tensor("bh", [D], F32, kind="ExternalInput").ap()
    idf = nc.dram_tensor("idf", [P, P], F32, kind="ExternalInput").ap()
    idh = nc.dram_tensor("idh", [P, P], FP16, kind="ExternalInput").ap()
    mkb = nc.dram_tensor("mkb", [P, 2 * P], F32, kind="ExternalInput").ap()
    zer = nc.dram_tensor("zer", [P, 2 * P], F32, kind="ExternalInput").ap()
    out_t = nc.dram_tensor("out_t", [2, S], F32, kind="ExternalOutput").ap()

    with tile.TileContext(nc) as tc:
        with (
            tc.tile_pool(name="persist", bufs=1) as persist,
            tc.tile_pool(name="work", bufs=6) as work,
            tc.tile_pool(name="psum", bufs=8, space="PSUM") as psum,
        ):
            # ---------------- persistent tiles ----------------
            w1_sb = persist.tile([P, 2, D], F32R)
            wq_sb = persist.tile([P, 2, D], F32R)
            wk_sb = persist.tile([P, 2, D], F32R)
            wv_sb = persist.tile([P, 2, D], F32R)
            wh_sb = persist.tile([P, 2, D], F32R)
            wo_sb = persist.tile([P, 2, 2], F32R)
            b1_sb = persist.tile([P, 2], F32)
            bh_sb = persist.tile([P, 2], F32)
            id_r = persist.tile([P, P], F32R)
            id_h = persist.tile([P, P], FP16)
            mk_r = persist.tile([P, 2 * P], F32R)

            xt_sb = persist.tile([P, 2, S], F32R)
            ht = persist.tile([P, 2, PADW], F32R)  # col = token + A
            qt = persist.tile([P, 2, S], F32R)
            kt = persist.tile([P, 2, PADW], F32R)  # col = token + A
            vs = persist.tile([P, NC + 1, D], FP16)  # tile t row p = token t*128+p-A
            att = persist.tile([P, 2, S], F32R)
            hid = persist.tile([P, 2, S], F32R)
            ot_sb = persist.tile([2, S], F32)

            def rearr(w):
                return w.rearrange("(k p) h -> p k h", p=P).bitcast(F32R)

            # ---- startup DMAs, in need-order; first-needed first ----
            nc.sync.dma_start(w1_sb[:], rearr(w1))
            nc.sync.dma_start(b1_sb[:], b1.rearrange("(hm p) -> p hm", p=P))
            nc.sync.dma_start(
                xt_sb[:, :, 0:512],
                xt.rearrange("(ko p) s -> p ko s", p=P)[:, :, 0:512].bitcast(F32R),
            )
            # second HWDGE ring (ACT) carries constants + pads in parallel
            nc.scalar.dma_start(id_r[:], idf.bitcast(F32R))
            nc.scalar.dma_start(id_h[:], idh)
            nc.scalar.dma_start(mk_r[:], mkb.bitcast(F32R))
            for ko in range(2):
                nc.scalar.dma_start(ht[:, ko, 0:A], zer[:, 0:A].bitcast(F32R))
                nc.scalar.dma_start(kt[:, ko, 0:A], zer[:, 0:A].bitcast(F32R))
                nc.scalar.dma_start(
                    ht[:, ko, S + A:PADW], zer[:, 0:PADW - S - A].bitcast(F32R)
                )
                nc.scalar.dma_start(
                    kt[:, ko, S + A:PADW], zer[:, 0:PADW - S - A].bitcast(F32R)
                )
            nc.sync.dma_start(wq_sb[:], rearr(wq))
            nc.sync.dma_start(wk_sb[:], rearr(wk))
            for t in range(1, 4):
                nc.sync.dma_start(
                    xt_sb[:, :, t * 512:(t + 1) * 512],
                    xt.rearrange("(ko p) s -> p ko s", p=P)[
                        :, :, t * 512:(t + 1) * 512
                    ].bitcast(F32R),
                )
            nc.scalar.dma_start(wv_sb[:], rearr(wv))
            nc.scalar.dma_start(wh_sb[:], rearr(wh))
            nc.scalar.dma_start(wo_sb[:], rearr(wo))
            nc.scalar.dma_start(bh_sb[:], bh.rearrange("(hm p) -> p hm", p=P))

            # ---------------- per-stripe phase bodies ----------------
            def p1_stripe(t):  # ht = relu(W1^T @ xt + b1), 512 tokens
                for hm in range(2):
                    ps = psum.tile([P, 512], F32, tag="bank")
                    for k in range(2):
                        nc.tensor.matmul(
                            ps[:], w1_sb[:, k, hm * P:(hm + 1) * P],
                            xt_sb[:, k, t * 512:(t + 1) * 512],
                            start=(k == 0), stop=(k == 1),
                        )
                    nc.scalar.activation(
                        ht[:, hm, A + t * 512:A + (t + 1) * 512], ps[:],
                        mybir.ActivationFunctionType.Relu, bias=b1_sb[:, hm:hm + 1],
                    )

            def p23_stripe(t):  # qt (ACT copy), kt (DVE copy)
                for hm in range(2):
                    psq = psum.tile([P, 512], F32, tag="bank")
                    for k in range(2):
                        nc.tensor.matmul(
                            psq[:], wq_sb[:, k, hm * P:(hm + 1) * P],
                            ht[:, k, A + t * 512:A + (t + 1) * 512],
                            start=(k == 0), stop=(k == 1),
                        )
                    nc.scalar.copy(qt[:, hm, t * 512:(t + 1) * 512], psq[:])
                    psk = psum.tile([P, 512], F32, tag="bank")
                    for k in range(2):
                        nc.tensor.matmul(
                            psk[:], wk_sb[:, k, hm * P:(hm + 1) * P],
                            ht[:, k, A + t * 512:A + (t + 1) * 512],
                            start=(k == 0), stop=(k == 1),
                        )
                    nc.vector.tensor_copy(
                        kt[:, hm, A + t * 512:A + (t + 1) * 512], psk[:]
                    )

            def p4_tile(v):  # shifted V tile (natural layout, fp16)
                psv = psum.tile([P, D], F32, tag="bank")
                for k in range(2):
                    nc.tensor.matmul(
                        psv[:], ht[:, k, v * P:(v + 1) * P], wv_sb[:, k, :],
                        start=(k == 0), stop=(k == 1),
                    )
                nc.vector.tensor_copy(vs[:, v, :], psv[:])

            def p5_pair(cp):  # band attention for chunks 2cp, 2cp+1
                psa = [psum.tile([P, 2 * P], F32, tag="bank", name=f"att{cp}_{fm}")
                       for fm in range(2)]
                for ci in range(2):
                    c = 2 * cp + ci
                    psl = psum.tile([P, 2 * P], F32, tag="bank", name="logit")
                    # additive band mask pre-loaded via identity matmul
                    nc.tensor.matmul(psl[:], id_r[:], mk_r[:], start=True, stop=False)
                    for k in range(2):
                        nc.tensor.matmul(
                            psl[:], qt[:, k, c * P:(c + 1) * P],
                            kt[:, k, c * P:c * P + 2 * P],
                            start=False, stop=(k == 1),
                        )
                    e = work.tile([P, 2 * P], FP16, tag="e")
                    den = work.tile([P, 1], F32, tag="den")
                    nc.scalar.activation(
                        e[:], psl[:], mybir.ActivationFunctionType.Exp,
                        scale=0.0625, accum_out=den[:],
                    )
                    rec = work.tile([P, 1], F32, tag="rec")
                    nc.vector.reciprocal(rec[:], den[:])
                    enb = work.tile([P, 2 * P], FP16, tag="enb")
                    nc.vector.tensor_scalar_mul(enb[:], e[:], rec[:])
                    pse = psum.tile([P, 2, P], FP16, tag="bank", name="etr")
                    for b in range(2):
                        nc.tensor.transpose(
                            pse[:, b, :], enb[:, b * P:(b + 1) * P], id_h[:]
                        )
                    et = work.tile([P, 2, P], FP16, tag="et")
                    nc.vector.tensor_copy(et[:], pse[:])
                    for fm in range(2):
                        for b in range(2):
                            nc.tensor.matmul(
                                psa[fm][:, ci * P:(ci + 1) * P],
                                vs[:, c + b, fm * P:(fm + 1) * P],
                                et[:, b, :],
                                start=(b == 0), stop=(b == 1),
                            )
                for fm in range(2):
                    nc.vector.tensor_copy(
                        att[:, fm, cp * 2 * P:(cp + 1) * 2 * P], psa[fm][:]
                    )

            def p6_stripe(u):  # hid = relu(Wh^T @ att + bh)
                for hm in range(2):
                    ps = psum.tile([P, 512], F32, tag="bank")
                    for k in range(2):
                        nc.tensor.matmul(
                            ps[:], wh_sb[:, k, hm * P:(hm + 1) * P],
                            att[:, k, u * 512:(u + 1) * 512],
                            start=(k == 0), stop=(k == 1),
                        )
                    nc.scalar.activation(
                        hid[:, hm, u * 512:(u + 1) * 512], ps[:],
                        mybir.ActivationFunctionType.Relu, bias=bh_sb[:, hm:hm + 1],
                    )

            def p7_stripe(u):  # out^T = Wo^T @ hid
                pso = psum.tile([2, 512], F32, tag="bank", name="outp")
                for k in range(2):
                    nc.tensor.matmul(
                        pso[:], wo_sb[:, k, :],
                        hid[:, k, u * 512:(u + 1) * 512],
                        start=(k == 0), stop=(k == 1),
                    )
                nc.scalar.copy(ot_sb[:, u * 512:(u + 1) * 512], pso[:])

            # ---------------- token-stripe wave ----------------
            # after stripe t of p1-p4, chunks up to 4t+2 have their inputs;
            # emit attention pairs as soon as both chunks are ready, and the
            # hid/out stripes as soon as their 4 chunks of att are copied.
            done_pairs = 0
            done_p6 = 0
            for t in range(4):
                p1_stripe(t)
                p23_stripe(t)
                for v in range(4 * t, 4 * t + 4):
                    p4_tile(v)
                if t == 3:
                    p4_tile(NC)
                ready_chunks = min(4 * t + 3, NC)  # chunks 0..ready-1 runnable
                while 2 * (done_pairs + 1) <= ready_chunks:
                    p5_pair(done_pairs)
                    done_pairs += 1
                while done_p6 < done_pairs // 2:
                    p6_stripe(done_p6)
                    p7_stripe(done_p6)
                    done_p6 += 1
            while done_pairs < NC // 2:
                p5_pair(done_pairs)
                done_pairs += 1
                while done_p6 < done_pairs // 2:
                    p6_stripe(done_p6)
                    p7_stripe(done_p6)
                    done_p6 += 1

            nc.sync.dma_start(out_t, ot_sb[:])

    nc.compile()
    return nc


def _get_nc():
    global _CACHED_NC
    if _CACHED_NC is None:
        _CACHED_NC = _build_nc()
    return _CACHED_NC


def _band_mask():
    j = np.arange(2 * P)[None, :]
    p = np.arange(P)[:, None]
    return np.where((j >= p) & (j <= p + 2 * A), 0.0, -1e9).astype(np.float32)


def kernel(x, W1, b1, Wq, Wk, Wv, Wh, bh, Wo, bo, **_unused):
    x = np.asarray(x, dtype=np.float32)
    W1 = np.asarray(W1, dtype=np.float32)
    Wq = np.asarray(Wq, dtype=np.float32)
    Wk = np.asarray(Wk, dtype=np.float32)
    Wv = np.asarray(Wv, dtype=np.float32)
    Wh = np.asarray(Wh, dtype=np.float32)
    Wo = np.asarray(Wo, dtype=np.float32)
    b1f = np.asarray(b1, dtype=np.float32).reshape(D)
    bhf = np.asarray(bh, dtype=np.float32).reshape(D)
    bof = np.asarray(bo, dtype=np.float32).reshape(2)
    zer = np.zeros((P, 2 * P), dtype=np.float32)
    idf = np.eye(P, dtype=np.float32)
    idh = np.eye(P, dtype=np.float16)
    mkb = _band_mask()

    nc = _get_nc()
    in_maps = []
    for b in range(NCORES):
        in_maps.append({
            "xt": np.ascontiguousarray(x[b].T),
            "w1": W1, "wq": Wq, "wk": Wk, "wv": Wv, "wh": Wh, "wo": Wo,
            "b1": b1f, "bh": bhf, "zer": zer,
            "idf": idf, "idh": idh, "mkb": mkb,
        })
    res = run_bass_kernel_spmd(nc, in_maps, core_ids=list(range(NCORES)))
    global _LAST_RESULTS
    _LAST_RESULTS = res
    out = np.stack(
        [res.results[b]["out_t"].T + bof[None, :] for b in range(NCORES)], axis=0
    )
    return out.astype(np.float32)


if __name__ == "__main__":
    rng = np.random.default_rng(0)
    ins = {
        "x": rng.standard_normal((8, S, D), dtype=np.float32),
        "W1": (rng.standard_normal((D, D), dtype=np.float32) / 16),
        "b1": np.zeros((1, 1, D), np.float32),
        "Wq": (rng.standard_normal((D, D), dtype=np.float32) / 16),
        "Wk": (rng.standard_normal((D, D), dtype=np.float32) / 16),
        "Wv": (rng.standard_normal((D, D), dtype=np.float32) / 16),
        "Wh": (rng.standard_normal((D, D), dtype=np.float32) / 16),
        "bh": np.zeros((1, 1, D), np.float32),
        "Wo": (rng.standard_normal((D, 2), dtype=np.float32) / 16),
        "bo": np.zeros((1, 1, 2), np.float32),
    }
    y = kernel(**ins)
    print("kernel output", y.shape, y.dtype, float(np.abs(y).max()))


# revision 9
# speedup vs baseline: 1.4811x; 1.3840x over previous
"""Trainium2 Bass kernel for windowed local self-attention MLP.

Reference computation (per batch b, S=2048 tokens, D=H=256, A=16, W=33):
    h   = relu(x @ W1 + b1)
    Q   = h @ Wq ; Khat = h @ Wk ; Vhat = h @ Wv        (windowed K/V are
          shifted views of Khat/Vhat -- the algebraic collapse of the
          reference's [B,S,W,H] window tensor)
    logit[s,m] = Q[s].Khat[s+A-m]/sqrt(H)  (zero outside [0,S), m=0..32)
    attn = softmax(logit) ; att[s] = sum_m attn[s,m] Vhat[s+A-m]
    out = relu(att @ Wh + bh) @ Wo + bo

Sharding: data-parallel over batch, one batch element per NeuronCore (B=8,
8 cores), weights replicated, no collectives.

Layout: activations feature-on-partition ([256=2x128, S]) so dense matmuls
contract over partitions; x is transposed host-side. Band attention runs per
128-token chunk over a 256-token window aligned to shifted (-A) 128-tiles.
The additive -1e9 band mask is pre-loaded into PSUM by an identity matmul,
the QK logits accumulate on top, exp runs with a fused row-sum (accum_out),
and a PE transpose of the normalized fp16 weights feeds [V-tile]^T @ [e]^T
fp16 matmuls. Dense matmuls use float32r (full PE rate, ~2e-4 rel err).
Emission is a token-stripe wave so DMA/PE/ACT/DVE overlap across phases.
Final projection is emitted transposed [2, S]; un-transposed and bo added
host-side.
"""
import sys

if "/opt/trn_rl_repo" not in sys.path:
    sys.path.insert(0, "/opt/trn_rl_repo")

import numpy as np

import concourse.mybir as mybir
import concourse.tile as tile
from concourse import bacc
from concourse.bass_utils import run_bass_kernel_spmd

P = 128
S = 2048  # tokens per core
D = 256  # model dim
A = 16  # half window
NC = 16  # token chunks per core
NCORES = 8

PADW = P * (NC + 1)  # 2176: padded token axis, col = token + A
F32 = mybir.dt.float32
F32R = mybir.dt.float32r
FP16 = mybir.dt.float16

_CACHED_NC = None
_LAST_RESULTS = None


def _build_nc():
    nc = bacc.Bacc(
        "TRN2",
        target_bir_lowering=False,
        debug=False,
        enable_asserts=False,
        num_devices=NCORES,
    )
    xt = nc.dram_tensor("xt", [D, S], F32, kind="ExternalInput").ap()
    w1 = nc.dram_tensor("w1", [D, D], F32, kind="ExternalInput").ap()
    wq = nc.dram_tensor("wq", [D, D], F32, kind="ExternalInput").ap()
    wk = nc.dram_tensor("wk", [D, D], F32, kind="ExternalInput").ap()
    wv = nc.dram_tensor("wv", [D, D], F32, kind="ExternalInput").ap()
    wh = nc.dram_tensor("wh", [D, D], F32, kind="ExternalInput").ap()
    wo = nc.dram_tensor("wo", [D, 2], F32, kind="ExternalInput").ap()
    b1 = nc.dram_tensor("b1", [D], F32, kind="ExternalInput").ap()
    bh = nc.dram_tensor("bh", [D], F32, kind="ExternalInput").ap()
    idf = nc.dram_tensor("idf", [P, P], F32, kind="ExternalInput").ap()
    idh = nc.dram_tensor("idh", [P, P], FP16, kind="ExternalInput").ap()
    mkb = nc.dram_tensor("mkb", [P, 4 * P], F32, kind="ExternalInput").ap()
    zer = nc.dram_tensor("zer", [P, 2 * P], F32, kind="ExternalInput").ap()
    out_t = nc.dram_tensor("out_t", [2, S], F32, kind="ExternalOutput").ap()

    with tile.TileContext(nc) as tc:
        with (
            tc.tile_pool(name="persist", bufs=1) as persist,
            tc.tile_pool(name="work", bufs=6) as work,
            tc.tile_pool(name="psum", bufs=8, space="PSUM") as psum,
        ):
            # ---------------- persistent tiles ----------------
            w1_sb = persist.tile([P, 2, D], F32R)
            wq_sb = persist.tile([P, 2, D], F32R)
            wk_sb = persist.tile([P, 2, D], F32R)
            wv_sb = persist.tile([P, 2, D], F32R)
            wh_sb = persist.tile([P, 2, D], F32R)
            wo_sb = persist.tile([P, 2, 2], F32R)
            b1_sb = persist.tile([P, 2], F32)
            bh_sb = persist.tile([P, 2], F32)
            id_r = persist.tile([P, P], F32R)
            id_h = persist.tile([P, P], FP16)
            mk_r = persist.tile([P, 4 * P], F32R)

            xt_sb = persist.tile([P, 2, S], F32R)
            ht = persist.tile([P, 2, PADW], F32R)  # col = token + A
            qt = persist.tile([P, 2, S], F32R)
            kt = persist.tile([P, 2, PADW], F32R)  # col = token + A
            vs = persist.tile([P, NC + 1, D], FP16)  # tile t row p = token t*128+p-A
            att = persist.tile([P, 2, S], F32R)
            hid = persist.tile([P, 2, S], F32R)
            ot_sb = persist.tile([2, S], F32)

            def rearr(w):
                return w.rearrange("(k p) h -> p k h", p=P).bitcast(F32R)

            # ---- startup DMAs, in need-order; first-needed first ----
            nc.sync.dma_start(w1_sb[:], rearr(w1))
            nc.sync.dma_start(b1_sb[:], b1.rearrange("(hm p) -> p hm", p=P))
            nc.sync.dma_start(
                xt_sb[:, :, 0:512],
                xt.rearrange("(ko p) s -> p ko s", p=P)[:, :, 0:512].bitcast(F32R),
            )
            nc.sync.dma_start(wq_sb[:], rearr(wq))
            nc.sync.dma_start(wk_sb[:], rearr(wk))
            for t in range(1, 4):
                nc.sync.dma_start(
                    xt_sb[:, :, t * 512:(t + 1) * 512],
                    xt.rearrange("(ko p) s -> p ko s", p=P)[
                        :, :, t * 512:(t + 1) * 512
                    ].bitcast(F32R),
                )
            # non-critical loads ride SWDGE on the otherwise-idle Pool engine
            nc.gpsimd.dma_start(wv_sb[:], rearr(wv))
            nc.gpsimd.dma_start(id_r[:], idf.bitcast(F32R))
            nc.gpsimd.dma_start(id_h[:], idh)
            nc.gpsimd.dma_start(mk_r[:], mkb.bitcast(F32R))
            for ko in range(2):
                nc.gpsimd.dma_start(ht[:, ko, 0:A], zer[:, 0:A].bitcast(F32R))
                nc.gpsimd.dma_start(kt[:, ko, 0:A], zer[:, 0:A].bitcast(F32R))
                nc.gpsimd.dma_start(
                    ht[:, ko, S + A:PADW], zer[:, 0:PADW - S - A].bitcast(F32R)
                )
                nc.gpsimd.dma_start(
                    kt[:, ko, S + A:PADW], zer[:, 0:PADW - S - A].bitcast(F32R)
                )
            nc.gpsimd.dma_start(wh_sb[:], rearr(wh))
            nc.gpsimd.dma_start(wo_sb[:], rearr(wo))
            nc.gpsimd.dma_start(bh_sb[:], bh.rearrange("(hm p) -> p hm", p=P))

            # ---------------- per-stripe phase bodies ----------------
            def p1_stripe(t):  # ht = relu(W1^T @ xt + b1), 512 tokens
                for hm in range(2):
                    ps = psum.tile([P, 512], F32, tag="bank")
                    for k in range(2):
                        nc.tensor.matmul(
                            ps[:], w1_sb[:, k, hm * P:(hm + 1) * P],
                            xt_sb[:, k, t * 512:(t + 1) * 512],
                            start=(k == 0), stop=(k == 1),
                        )
                    nc.scalar.activation(
                        ht[:, hm, A + t * 512:A + (t + 1) * 512], ps[:],
                        mybir.ActivationFunctionType.Relu, bias=b1_sb[:, hm:hm + 1],
                    )

            def p23_stripe(t):  # qt (ACT copy), kt (DVE copy)
                for hm in range(2):
                    psq = psum.tile([P, 512], F32, tag="bank")
                    for k in range(2):
                        nc.tensor.matmul(
                            psq[:], wq_sb[:, k, hm * P:(hm + 1) * P],
                            ht[:, k, A + t * 512:A + (t + 1) * 512],
                            start=(k == 0), stop=(k == 1),
                        )
                    nc.scalar.copy(qt[:, hm, t * 512:(t + 1) * 512], psq[:])
                    psk = psum.tile([P, 512], F32, tag="bank")
                    for k in range(2):
                        nc.tensor.matmul(
                            psk[:], wk_sb[:, k, hm * P:(hm + 1) * P],
                            ht[:, k, A + t * 512:A + (t + 1) * 512],
                            start=(k == 0), stop=(k == 1),
                        )
                    nc.vector.tensor_copy(
                        kt[:, hm, A + t * 512:A + (t + 1) * 512], psk[:]
                    )

            def p4_tile(v):  # shifted V tile (natural layout, fp16)
                psv = psum.tile([P, D], F32, tag="bank")
                for k in range(2):
                    nc.tensor.matmul(
                        psv[:], ht[:, k, v * P:(v + 1) * P], wv_sb[:, k, :],
                        start=(k == 0), stop=(k == 1),
                    )
                nc.vector.tensor_copy(vs[:, v, :], psv[:])

            # ---- band attention, software-pipelined per chunk-pair ----
            # stage A (PE): mask-init + QK logits for both chunks into one bank
            # stage B (ACT/DVE): exp+rowsum, recip, normalize (fp16)
            # stage C (PE/DVE): transpose weights, apply V, copy att out
            pair_state = {}

            def p5_logits(cp):
                psl = psum.tile([P, 4 * P], F32, tag="bank", name="logit")
                nc.tensor.matmul(psl[:], id_r[:], mk_r[:], start=True, stop=False)
                for ci in range(2):
                    c = 2 * cp + ci
                    for k in range(2):
                        nc.tensor.matmul(
                            psl[:, ci * 2 * P:(ci + 1) * 2 * P],
                            qt[:, k, c * P:(c + 1) * P],
                            kt[:, k, c * P:c * P + 2 * P],
                            start=False, stop=(ci == 1 and k == 1),
                        )
                pair_state[cp] = psl

            def p5_softmax(cp):
                psl = pair_state.pop(cp)
                enb = work.tile([P, 4 * P], FP16, tag="enb")
                for ci in range(2):
                    sl = slice(ci * 2 * P, (ci + 1) * 2 * P)
                    e = work.tile([P, 2 * P], FP16, tag="e")
                    den = work.tile([P, 1], F32, tag="den")
                    nc.scalar.activation(
                        e[:], psl[:, sl], mybir.ActivationFunctionType.Exp,
                        scale=0.0625, accum_out=den[:],
                    )
                    rec = work.tile([P, 1], F32, tag="rec")
                    nc.vector.reciprocal(rec[:], den[:])
                    nc.vector.tensor_scalar_mul(enb[:, sl], e[:], rec[:])
                pair_state[("enb", cp)] = enb

            def p5_apply(cp):
                enb = pair_state.pop(("enb", cp))
                pse = psum.tile([P, 4, P], FP16, tag="bank", name="etr")
                for b in range(4):
                    nc.tensor.transpose(
                        pse[:, b, :], enb[:, b * P:(b + 1) * P], id_h[:]
                    )
                et = work.tile([P, 4, P], FP16, tag="et")
                nc.vector.tensor_copy(et[:], pse[:])
                psa = [psum.tile([P, 2 * P], F32, tag="bank", name=f"att{cp % 2}_{fm}")
                       for fm in range(2)]
                for ci in range(2):
                    c = 2 * cp + ci
                    for fm in range(2):
                        for b in range(2):
                            nc.tensor.matmul(
                                psa[fm][:, ci * P:(ci + 1) * P],
                                vs[:, c + b, fm * P:(fm + 1) * P],
                                et[:, 2 * ci + b, :],
                                start=(b == 0), stop=(b == 1),
                            )
                for fm in range(2):
                    nc.vector.tensor_copy(
                        att[:, fm, cp * 2 * P:(cp + 1) * 2 * P], psa[fm][:]
                    )

            def p6_stripe(u):  # hid = relu(Wh^T @ att + bh)
                for hm in range(2):
                    ps = psum.tile([P, 512], F32, tag="bank")
                    for k in range(2):
                        nc.tensor.matmul(
                            ps[:], wh_sb[:, k, hm * P:(hm + 1) * P],
                            att[:, k, u * 512:(u + 1) * 512],
                            start=(k == 0), stop=(k == 1),
                        )
                    nc.scalar.activation(
                        hid[:, hm, u * 512:(u + 1) * 512], ps[:],
                        mybir.ActivationFunctionType.Relu, bias=bh_sb[:, hm:hm + 1],
                    )

            def p7_stripe(u):  # out^T = Wo^T @ hid
                pso = psum.tile([2, 512], F32, tag="bank", name="outp")
                for k in range(2):
                    nc.tensor.matmul(
                        pso[:], wo_sb[:, k, :],
                        hid[:, k, u * 512:(u + 1) * 512],
                        start=(k == 0), stop=(k == 1),
                    )
                nc.scalar.copy(ot_sb[:, u * 512:(u + 1) * 512], pso[:])

            # ---------------- token-stripe wave + pipelined attention -------
            # Stage skew keeps each engine's stream from blocking on the
            # cross-engine round trip: logits(cp) run ~2 pair-stages ahead of
            # apply(cp).
            rounds = NC // 2
            lg = sm = ap = p6u = 0

            def flush_p6():
                nonlocal p6u
                while p6u < ap // 2:
                    p6_stripe(p6u)
                    p7_stripe(p6u)
                    p6u += 1

            for t in range(4):
                p1_stripe(t)
                p23_stripe(t)
                for v in range(4 * t, 4 * t + 4):
                    p4_tile(v)
                if t == 3:
                    p4_tile(NC)
                max_chunk = 4 * t + 2 if t < 3 else NC - 1
                max_lg = (max_chunk - 1) // 2
                max_ap = (4 * t + 1) // 2 if t < 3 else rounds - 1
                while lg <= max_lg:
                    p5_logits(lg)
                    lg += 1
                    if sm < lg - 1:
                        p5_softmax(sm)
                        sm += 1
                    if ap < sm - 1 and ap <= max_ap:
                        p5_apply(ap)
                        ap += 1
                        flush_p6()
            while sm < rounds:
                p5_softmax(sm)
                sm += 1
                while ap < sm - 1:
                    p5_apply(ap)
                    ap += 1
                    flush_p6()
            while ap < rounds:
                p5_apply(ap)
                ap += 1
                flush_p6()

            nc.sync.dma_start(out_t, ot_sb[:])

    nc.compile()
    return nc


def _get_nc():
    global _CACHED_NC
    if _CACHED_NC is None:
        _CACHED_NC = _build_nc()
    return _CACHED_NC


def _band_mask():
    j = np.arange(2 * P)[None, :]
    p = np.arange(P)[:, None]
    m = np.where((j >= p) & (j <= p + 2 * A), 0.0, -1e9).astype(np.float32)
    return np.tile(m, (1, 2))


def kernel(x, W1, b1, Wq, Wk, Wv, Wh, bh, Wo, bo, **_unused):
    x = np.asarray(x, dtype=np.float32)
    W1 = np.asarray(W1, dtype=np.float32)
    Wq = np.asarray(Wq, dtype=np.float32)
    Wk = np.asarray(Wk, dtype=np.float32)
    Wv = np.asarray(Wv, dtype=np.float32)
    Wh = np.asarray(Wh, dtype=np.float32)
    Wo = np.asarray(Wo, dtype=np.float32)
    b1f = np.asarray(b1, dtype=np.float32).reshape(D)
    bhf = np.asarray(bh, dtype=np.float32).reshape(D)
    bof = np.asarray(bo, dtype=np.float32).reshape(2)
    zer = np.zeros((P, 2 * P), dtype=np.float32)
    idf = np.eye(P, dtype=np.float32)
    idh = np.eye(P, dtype=np.float16)
    mkb = _band_mask()

    nc = _get_nc()
    in_maps = []
    for b in range(NCORES):
        in_maps.append({
            "xt": np.ascontiguousarray(x[b].T),
            "w1": W1, "wq": Wq, "wk": Wk, "wv": Wv, "wh": Wh, "wo": Wo,
            "b1": b1f, "bh": bhf, "zer": zer,
            "idf": idf, "idh": idh, "mkb": mkb,
        })
    res = run_bass_kernel_spmd(nc, in_maps, core_ids=list(range(NCORES)))
    global _LAST_RESULTS
    _LAST_RESULTS = res
    out = np.stack(
        [res.results[b]["out_t"].T + bof[None, :] for b in range(NCORES)], axis=0
    )
    return out.astype(np.float32)


if __name__ == "__main__":
    rng = np.random.default_rng(0)
    ins = {
        "x": rng.standard_normal((8, S, D), dtype=np.float32),
        "W1": (rng.standard_normal((D, D), dtype=np.float32) / 16),
        "b1": np.zeros((1, 1, D), np.float32),
        "Wq": (rng.standard_normal((D, D), dtype=np.float32) / 16),
        "Wk": (rng.standard_normal((D, D), dtype=np.float32) / 16),
        "Wv": (rng.standard_normal((D, D), dtype=np.float32) / 16),
        "Wh": (rng.standard_normal((D, D), dtype=np.float32) / 16),
        "bh": np.zeros((1, 1, D), np.float32),
        "Wo": (rng.standard_normal((D, 2), dtype=np.float32) / 16),
        "bo": np.zeros((1, 1, 2), np.float32),
    }
    y = kernel(**ins)
    print("kernel output", y.shape, y.dtype, float(np.abs(y).max()))


# revision 10
# speedup vs baseline: 1.5418x; 1.0410x over previous
"""Trainium2 Bass kernel for windowed local self-attention MLP.

Reference computation (per batch b, S=2048 tokens, D=H=256, A=16, W=33):
    h   = relu(x @ W1 + b1)
    Q   = h @ Wq ; Khat = h @ Wk ; Vhat = h @ Wv        (windowed K/V are
          shifted views of Khat/Vhat -- the algebraic collapse of the
          reference's [B,S,W,H] window tensor)
    logit[s,m] = Q[s].Khat[s+A-m]/sqrt(H)  (zero outside [0,S), m=0..32)
    attn = softmax(logit) ; att[s] = sum_m attn[s,m] Vhat[s+A-m]
    out = relu(att @ Wh + bh) @ Wo + bo

Sharding: data-parallel over batch, one batch element per NeuronCore (B=8,
8 cores), weights replicated, no collectives.

Layout: activations feature-on-partition ([256=2x128, S]) so dense matmuls
contract over partitions; x is transposed host-side. Band attention runs per
128-token chunk over a 256-token window aligned to shifted (-A) 128-tiles.
The additive -1e9 band mask is pre-loaded into PSUM by an identity matmul,
the QK logits accumulate on top, exp runs with a fused row-sum (accum_out),
and a PE transpose of the normalized fp16 weights feeds [V-tile]^T @ [e]^T
fp16 matmuls. Dense matmuls use float32r (full PE rate, ~2e-4 rel err).
Emission is a token-stripe wave so DMA/PE/ACT/DVE overlap across phases.
Final projection is emitted transposed [2, S]; un-transposed and bo added
host-side.
"""
import sys

if "/opt/trn_rl_repo" not in sys.path:
    sys.path.insert(0, "/opt/trn_rl_repo")

import numpy as np

import concourse.mybir as mybir
import concourse.tile as tile
from concourse import bacc
from concourse.bass_utils import run_bass_kernel_spmd

P = 128
S = 2048  # tokens per core
D = 256  # model dim
A = 16  # half window
NC = 16  # token chunks per core
NCORES = 8

PADW = P * (NC + 1)  # 2176: padded token axis, col = token + A
WINW = P + 2 * A  # 160: per-chunk attention window
F32 = mybir.dt.float32
F32R = mybir.dt.float32r
FP16 = mybir.dt.float16

_CACHED_NC = None
_LAST_RESULTS = None


def _build_nc():
    nc = bacc.Bacc(
        "TRN2",
        target_bir_lowering=False,
        debug=False,
        enable_asserts=False,
        num_devices=NCORES,
    )
    xt = nc.dram_tensor("xt", [D, S], F32, kind="ExternalInput").ap()
    w1 = nc.dram_tensor("w1", [D, D], F32, kind="ExternalInput").ap()
    wq = nc.dram_tensor("wq", [D, D], F32, kind="ExternalInput").ap()
    wk = nc.dram_tensor("wk", [D, D], F32, kind="ExternalInput").ap()
    wv = nc.dram_tensor("wv", [D, D], F32, kind="ExternalInput").ap()
    wh = nc.dram_tensor("wh", [D, D], F32, kind="ExternalInput").ap()
    wo = nc.dram_tensor("wo", [D, 2], F32, kind="ExternalInput").ap()
    b1 = nc.dram_tensor("b1", [D], F32, kind="ExternalInput").ap()
    bh = nc.dram_tensor("bh", [D], F32, kind="ExternalInput").ap()
    idh = nc.dram_tensor("idh", [P, P], FP16, kind="ExternalInput").ap()
    mkb = nc.dram_tensor("mkb", [P, 2 * WINW], FP16, kind="ExternalInput").ap()
    zer = nc.dram_tensor("zer", [P, 2 * P], F32, kind="ExternalInput").ap()
    zerh = nc.dram_tensor("zerh", [P, 2 * P], FP16, kind="ExternalInput").ap()
    out_t = nc.dram_tensor("out_t", [2, S], F32, kind="ExternalOutput").ap()

    with tile.TileContext(nc) as tc:
        with (
            tc.tile_pool(name="persist", bufs=1) as persist,
            tc.tile_pool(name="work", bufs=6) as work,
            tc.tile_pool(name="psum", bufs=8, space="PSUM") as psum,
        ):
            # ---------------- persistent tiles ----------------
            w1_sb = persist.tile([P, 2, D], F32R)
            wq_sb = persist.tile([P, 2, D], F32R)
            wk_sb = persist.tile([P, 2, D], F32R)
            wv_sb = persist.tile([P, 2, D], F32R)
            wh_sb = persist.tile([P, 2, D], F32R)
            wo_sb = persist.tile([P, 2, 2], F32R)
            b1_sb = persist.tile([P, 2], F32)
            bh_sb = persist.tile([P, 2], F32)
            id_h = persist.tile([P, P], FP16)
            mk_h = persist.tile([P, 2 * WINW], FP16)

            xt_sb = persist.tile([P, 2, S], F32R)
            zf16 = zerh
            ht = persist.tile([P, 2, PADW], F32R)  # col = token + A
            qt = persist.tile([P, 2, S], FP16)
            kt = persist.tile([P, 2, PADW], FP16)  # col = token + A
            vs = persist.tile([P, NC + 1, D], FP16)  # tile t row p = token t*128+p-A
            att = persist.tile([P, 2, S], F32R)
            hid = persist.tile([P, 2, S], F32R)
            ot_sb = persist.tile([2, S], F32)

            def rearr(w):
                return w.rearrange("(k p) h -> p k h", p=P).bitcast(F32R)

            # ---- startup DMAs, in need-order; first-needed first ----
            nc.sync.dma_start(
                xt_sb[:, :, 0:512],
                xt.rearrange("(ko p) s -> p ko s", p=P)[:, :, 0:512].bitcast(F32R),
            )
            nc.sync.dma_start(w1_sb[:], rearr(w1))
            nc.sync.dma_start(b1_sb[:], b1.rearrange("(hm p) -> p hm", p=P))
            nc.sync.dma_start(wq_sb[:], rearr(wq))
            nc.sync.dma_start(wk_sb[:], rearr(wk))
            nc.sync.dma_start(
                xt_sb[:, :, 512:S],
                xt.rearrange("(ko p) s -> p ko s", p=P)[:, :, 512:S].bitcast(F32R),
            )
            # non-critical loads ride SWDGE on the otherwise-idle Pool engine
            nc.gpsimd.dma_start(wv_sb[:], rearr(wv))
            nc.gpsimd.dma_start(id_h[:], idh)
            nc.gpsimd.dma_start(mk_h[:], mkb)
            for ko in range(2):
                nc.gpsimd.dma_start(ht[:, ko, 0:A], zer[:, 0:A].bitcast(F32R))
                nc.gpsimd.dma_start(kt[:, ko, 0:A], zf16[:, 0:A])
                nc.gpsimd.dma_start(
                    ht[:, ko, S + A:PADW], zer[:, 0:PADW - S - A].bitcast(F32R)
                )
                nc.gpsimd.dma_start(kt[:, ko, S + A:PADW], zf16[:, 0:PADW - S - A])
            nc.gpsimd.dma_start(wh_sb[:], rearr(wh))
            nc.gpsimd.dma_start(wo_sb[:], rearr(wo))
            nc.gpsimd.dma_start(bh_sb[:], bh.rearrange("(hm p) -> p hm", p=P))

            # ---------------- per-stripe phase bodies ----------------
            def p1_stripe(t):  # ht = relu(W1^T @ xt + b1), 512 tokens
                for hm in range(2):
                    ps = psum.tile([P, 512], F32, tag="bank")
                    for k in range(2):
                        nc.tensor.matmul(
                            ps[:], w1_sb[:, k, hm * P:(hm + 1) * P],
                            xt_sb[:, k, t * 512:(t + 1) * 512],
                            start=(k == 0), stop=(k == 1),
                        )
                    nc.scalar.activation(
                        ht[:, hm, A + t * 512:A + (t + 1) * 512], ps[:],
                        mybir.ActivationFunctionType.Relu, bias=b1_sb[:, hm:hm + 1],
                    )

            def p23_stripe(t):  # qt (ACT copy), kt (DVE copy)
                for hm in range(2):
                    psq = psum.tile([P, 512], F32, tag="bank")
                    for k in range(2):
                        nc.tensor.matmul(
                            psq[:], wq_sb[:, k, hm * P:(hm + 1) * P],
                            ht[:, k, A + t * 512:A + (t + 1) * 512],
                            start=(k == 0), stop=(k == 1),
                        )
                    nc.scalar.copy(qt[:, hm, t * 512:(t + 1) * 512], psq[:])
                    psk = psum.tile([P, 512], F32, tag="bank")
                    for k in range(2):
                        nc.tensor.matmul(
                            psk[:], wk_sb[:, k, hm * P:(hm + 1) * P],
                            ht[:, k, A + t * 512:A + (t + 1) * 512],
                            start=(k == 0), stop=(k == 1),
                        )
                    nc.vector.tensor_copy(
                        kt[:, hm, A + t * 512:A + (t + 1) * 512], psk[:]
                    )

            def p4_tile(v):  # shifted V tile (natural layout, fp16)
                psv = psum.tile([P, D], F32, tag="bank")
                for k in range(2):
                    nc.tensor.matmul(
                        psv[:], ht[:, k, v * P:(v + 1) * P], wv_sb[:, k, :],
                        start=(k == 0), stop=(k == 1),
                    )
                nc.vector.tensor_copy(vs[:, v, :], psv[:])

            # ---- band attention, software-pipelined per chunk-pair ----
            # stage A (PE): mask-init + QK logits for both chunks into one bank
            # stage B (ACT/DVE): exp+rowsum, recip, normalize (fp16)
            # stage C (PE/DVE): transpose weights, apply V, copy att out
            pair_state = {}

            def p5_logits(cp):
                psl = psum.tile([P, 2 * WINW], F32, tag="bank", name="logit")
                nc.tensor.matmul(psl[:], id_h[:], mk_h[:], start=True, stop=False)
                for ci in range(2):
                    c = 2 * cp + ci
                    for k in range(2):
                        nc.tensor.matmul(
                            psl[:, ci * WINW:(ci + 1) * WINW],
                            qt[:, k, c * P:(c + 1) * P],
                            kt[:, k, c * P:c * P + WINW],
                            start=False, stop=(ci == 1 and k == 1),
                        )
                pair_state[cp] = psl

            def p5_softmax(cp):
                psl = pair_state.pop(cp)
                enb = work.tile([P, 2 * WINW], FP16, tag="enb")
                for ci in range(2):
                    sl = slice(ci * WINW, (ci + 1) * WINW)
                    e = work.tile([P, WINW], FP16, tag="e")
                    den = work.tile([P, 1], F32, tag="den")
                    nc.scalar.activation(
                        e[:], psl[:, sl], mybir.ActivationFunctionType.Exp,
                        scale=0.0625, accum_out=den[:],
                    )
                    rec = work.tile([P, 1], F32, tag="rec")
                    nc.vector.reciprocal(rec[:], den[:])
                    nc.vector.tensor_scalar_mul(enb[:, sl], e[:], rec[:])
                pair_state[("enb", cp)] = enb

            def p5_apply(cp):
                enb = pair_state.pop(("enb", cp))
                pse = psum.tile([P, 4, P], FP16, tag="bank", name="etr")
                for ci in range(2):
                    nc.tensor.transpose(
                        pse[:, 2 * ci, :],
                        enb[:, ci * WINW:ci * WINW + P], id_h[:]
                    )
                    nc.tensor.transpose(
                        pse[0:2 * A, 2 * ci + 1, :],
                        enb[:, ci * WINW + P:(ci + 1) * WINW], id_h[:]
                    )
                et = work.tile([P, 4, P], FP16, tag="et")
                nc.vector.tensor_copy(et[:], pse[:])
                psa = [psum.tile([P, 2 * P], F32, tag="bank", name=f"att{cp % 2}_{fm}")
                       for fm in range(2)]
                for ci in range(2):
                    c = 2 * cp + ci
                    for fm in range(2):
                        nc.tensor.matmul(
                            psa[fm][:, ci * P:(ci + 1) * P],
                            vs[:, c, fm * P:(fm + 1) * P],
                            et[:, 2 * ci, :],
                            start=True, stop=False,
                        )
                        nc.tensor.matmul(
                            psa[fm][:, ci * P:(ci + 1) * P],
                            vs[0:2 * A, c + 1, fm * P:(fm + 1) * P],
                            et[0:2 * A, 2 * ci + 1, :],
                            start=False, stop=True,
                        )
                for fm in range(2):
                    nc.vector.tensor_copy(
                        att[:, fm, cp * 2 * P:(cp + 1) * 2 * P], psa[fm][:]
                    )

            def p6_stripe(u):  # hid = relu(Wh^T @ att + bh)
                for hm in range(2):
                    ps = psum.tile([P, 512], F32, tag="bank")
                    for k in range(2):
                        nc.tensor.matmul(
                            ps[:], wh_sb[:, k, hm * P:(hm + 1) * P],
                            att[:, k, u * 512:(u + 1) * 512],
                            start=(k == 0), stop=(k == 1),
                        )
                    nc.scalar.activation(
                        hid[:, hm, u * 512:(u + 1) * 512], ps[:],
                        mybir.ActivationFunctionType.Relu, bias=bh_sb[:, hm:hm + 1],
                    )

            def p7_stripe(u):  # out^T = Wo^T @ hid
                pso = psum.tile([2, 512], F32, tag="bank", name="outp")
                for k in range(2):
                    nc.tensor.matmul(
                        pso[:], wo_sb[:, k, :],
                        hid[:, k, u * 512:(u + 1) * 512],
                        start=(k == 0), stop=(k == 1),
                    )
                nc.scalar.copy(ot_sb[:, u * 512:(u + 1) * 512], pso[:])

            # ---------------- token-stripe wave + pipelined attention -------
            # Stage skew keeps each engine's stream from blocking on the
            # cross-engine round trip: logits(cp) run ~2 pair-stages ahead of
            # apply(cp).
            rounds = NC // 2
            lg = sm = ap = p6u = 0

            def flush_p6():
                nonlocal p6u
                while p6u < ap // 2:
                    p6_stripe(p6u)
                    p7_stripe(p6u)
                    p6u += 1

            for t in range(4):
                p1_stripe(t)
                p23_stripe(t)
                for v in range(4 * t, 4 * t + 4):
                    p4_tile(v)
                if t == 3:
                    p4_tile(NC)
                max_chunk = 4 * t + 2 if t < 3 else NC - 1
                max_lg = (max_chunk - 1) // 2
                max_ap = (4 * t + 1) // 2 if t < 3 else rounds - 1
                while lg <= max_lg:
                    p5_logits(lg)
                    lg += 1
                    if sm < lg - 1:
                        p5_softmax(sm)
                        sm += 1
                    if ap < sm - 1 and ap <= max_ap:
                        p5_apply(ap)
                        ap += 1
                        flush_p6()
            while sm < rounds:
                p5_softmax(sm)
                sm += 1
                while ap < sm - 1:
                    p5_apply(ap)
                    ap += 1
                    flush_p6()
            while ap < rounds:
                p5_apply(ap)
                ap += 1
                flush_p6()

            nc.sync.dma_start(out_t, ot_sb[:])

    nc.compile()
    return nc


def _get_nc():
    global _CACHED_NC
    if _CACHED_NC is None:
        _CACHED_NC = _build_nc()
    return _CACHED_NC


def _band_mask():
    j = np.arange(WINW)[None, :]
    p = np.arange(P)[:, None]
    m = np.where((j >= p) & (j <= p + 2 * A), 0.0, -60000.0).astype(np.float16)
    return np.tile(m, (1, 2))


def kernel(x, W1, b1, Wq, Wk, Wv, Wh, bh, Wo, bo, **_unused):
    x = np.asarray(x, dtype=np.float32)
    W1 = np.asarray(W1, dtype=np.float32)
    Wq = np.asarray(Wq, dtype=np.float32)
    Wk = np.asarray(Wk, dtype=np.float32)
    Wv = np.asarray(Wv, dtype=np.float32)
    Wh = np.asarray(Wh, dtype=np.float32)
    Wo = np.asarray(Wo, dtype=np.float32)
    b1f = np.asarray(b1, dtype=np.float32).reshape(D)
    bhf = np.asarray(bh, dtype=np.float32).reshape(D)
    bof = np.asarray(bo, dtype=np.float32).reshape(2)
    zer = np.zeros((P, 2 * P), dtype=np.float32)
    zerh = np.zeros((P, 2 * P), dtype=np.float16)
    idh = np.eye(P, dtype=np.float16)
    mkb = _band_mask()

    nc = _get_nc()
    in_maps = []
    for b in range(NCORES):
        in_maps.append({
            "xt": np.ascontiguousarray(x[b].T),
            "w1": W1, "wq": Wq, "wk": Wk, "wv": Wv, "wh": Wh, "wo": Wo,
            "b1": b1f, "bh": bhf, "zer": zer, "zerh": zerh,
            "idh": idh, "mkb": mkb,
        })
    res = run_bass_kernel_spmd(nc, in_maps, core_ids=list(range(NCORES)))
    global _LAST_RESULTS
    _LAST_RESULTS = res
    out = np.stack(
        [res.results[b]["out_t"].T + bof[None, :] for b in range(NCORES)], axis=0
    )
    return out.astype(np.float32)


if __name__ == "__main__":
    rng = np.random.default_rng(0)
    ins = {
        "x": rng.standard_normal((8, S, D), dtype=np.float32),
        "W1": (rng.standard_normal((D, D), dtype=np.float32) / 16),
        "b1": np.zeros((1, 1, D), np.float32),
        "Wq": (rng.standard_normal((D, D), dtype=np.float32) / 16),
        "Wk": (rng.standard_normal((D, D), dtype=np.float32) / 16),
        "Wv": (rng.standard_normal((D, D), dtype=np.float32) / 16),
        "Wh": (rng.standard_normal((D, D), dtype=np.float32) / 16),
        "bh": np.zeros((1, 1, D), np.float32),
        "Wo": (rng.standard_normal((D, 2), dtype=np.float32) / 16),
        "bo": np.zeros((1, 1, 2), np.float32),
    }
    y = kernel(**ins)
    print("kernel output", y.shape, y.dtype, float(np.abs(y).max()))


# revision 14
# speedup vs baseline: 1.8027x; 1.1692x over previous
"""Trainium2 Bass kernel for windowed local self-attention MLP.

Reference computation (per batch b, S=2048 tokens, D=H=256, A=16, W=33):
    h   = relu(x @ W1 + b1)
    Q   = h @ Wq ; Khat = h @ Wk ; Vhat = h @ Wv        (windowed K/V are
          shifted views of Khat/Vhat -- the algebraic collapse of the
          reference's [B,S,W,H] window tensor)
    logit[s,m] = Q[s].Khat[s+A-m]/sqrt(H)  (zero outside [0,S), m=0..32)
    attn = softmax(logit) ; att[s] = sum_m attn[s,m] Vhat[s+A-m]
    out = relu(att @ Wh + bh) @ Wo + bo

Sharding: data-parallel over batch, one batch element per NeuronCore (B=8,
8 cores), weights replicated, no collectives.

Layout: activations feature-on-partition ([256=2x128, S]) so dense matmuls
contract over partitions; x is transposed host-side. Band attention runs per
128-token chunk over a 256-token window aligned to shifted (-A) 128-tiles.
The additive -1e9 band mask is pre-loaded into PSUM by an identity matmul,
the QK logits accumulate on top, exp runs with a fused row-sum (accum_out),
and a PE transpose of the normalized fp16 weights feeds [V-tile]^T @ [e]^T
fp16 matmuls. Dense matmuls use float32r (full PE rate, ~2e-4 rel err).
Emission is a token-stripe wave so DMA/PE/ACT/DVE overlap across phases.
Final projection is emitted transposed [2, S]; un-transposed and bo added
host-side.
"""
import sys

if "/opt/trn_rl_repo" not in sys.path:
    sys.path.insert(0, "/opt/trn_rl_repo")

import numpy as np

import concourse.mybir as mybir
import concourse.tile as tile
from concourse import bacc
from concourse.bass_utils import run_bass_kernel_spmd

P = 128
S = 2048  # tokens per core
D = 256  # model dim
A = 16  # half window
NC = 16  # token chunks per core
NCORES = 8

PADW = P * (NC + 1)  # 2176: padded token axis, col = token + A
WINW = P + 2 * A  # 160: per-chunk attention window
F32 = mybir.dt.float32
F32R = mybir.dt.float32r
FP16 = mybir.dt.float16

_CACHED_NC = None
_LAST_RESULTS = None


def _build_nc():
    nc = bacc.Bacc(
        "TRN2",
        target_bir_lowering=False,
        debug=False,
        enable_asserts=False,
        num_devices=NCORES,
    )
    xt = nc.dram_tensor("xt", [D, S], F32, kind="ExternalInput").ap()
    w1 = nc.dram_tensor("w1", [D, D], F32, kind="ExternalInput").ap()
    wq = nc.dram_tensor("wq", [D, D], F32, kind="ExternalInput").ap()
    wv = nc.dram_tensor("wv", [D, D], F32, kind="ExternalInput").ap()
    wo = nc.dram_tensor("wo", [D, 2], F32, kind="ExternalInput").ap()
    b1 = nc.dram_tensor("b1", [D], F32, kind="ExternalInput").ap()
    bh = nc.dram_tensor("bh", [D], F32, kind="ExternalInput").ap()
    idh = nc.dram_tensor("idh", [P, P], FP16, kind="ExternalInput").ap()
    mkb = nc.dram_tensor("mkb", [P, 2 * WINW], FP16, kind="ExternalInput").ap()
    zer = nc.dram_tensor("zer", [P, 2 * P], F32, kind="ExternalInput").ap()
    zerh = nc.dram_tensor("zerh", [P, 2 * P], FP16, kind="ExternalInput").ap()
    out_t = nc.dram_tensor("out_t", [2, S], F32, kind="ExternalOutput").ap()

    with tile.TileContext(nc) as tc:
        with (
            tc.tile_pool(name="persist", bufs=1) as persist,
            tc.tile_pool(name="work", bufs=6) as work,
            tc.tile_pool(name="psum", bufs=8, space="PSUM") as psum,
        ):
            # ---------------- persistent tiles ----------------
            w1_sb = persist.tile([P, 2, D], F32R)
            wq_sb = persist.tile([P, 2, D], F32R)
            wv_sb = persist.tile([P, 2, D], F32R)
            wo_sb = persist.tile([P, 2, 2], F32R)
            b1_sb = persist.tile([P, 2], F32)
            bh_sb = persist.tile([P, 2], F32)
            id_h = persist.tile([P, P], FP16)
            mk_h = persist.tile([P, 2 * WINW], FP16)

            xt_sb = persist.tile([P, 2, S], F32R)
            zf16 = zerh
            ht = persist.tile([P, 2, PADW], F32R)  # col = token + A
            qt = persist.tile([P, 2, S], FP16)
            kt = persist.tile([P, 2, PADW], FP16)  # col = token + A
            vs = persist.tile([P, NC + 1, D], FP16)  # tile t row p = token t*128+p-A
            hid = persist.tile([P, 2, S], F32R)
            ot_sb = persist.tile([2, S], F32)

            def rearr(w):
                return w.rearrange("(k p) h -> p k h", p=P).bitcast(F32R)

            # ---- startup DMAs, in need-order; first-needed first ----
            nc.sync.dma_start(
                xt_sb[:, :, 0:512],
                xt.rearrange("(ko p) s -> p ko s", p=P)[:, :, 0:512].bitcast(F32R),
            )
            nc.sync.dma_start(w1_sb[:], rearr(w1))
            nc.sync.dma_start(b1_sb[:], b1.rearrange("(hm p) -> p hm", p=P))
            nc.sync.dma_start(wq_sb[:], rearr(wq))
            nc.sync.dma_start(
                xt_sb[:, :, 512:S],
                xt.rearrange("(ko p) s -> p ko s", p=P)[:, :, 512:S].bitcast(F32R),
            )
            # non-critical loads ride SWDGE on the otherwise-idle Pool engine
            nc.gpsimd.dma_start(wv_sb[:], rearr(wv))
            nc.gpsimd.dma_start(id_h[:], idh)
            nc.gpsimd.dma_start(mk_h[:], mkb)
            for ko in range(2):
                nc.gpsimd.dma_start(ht[:, ko, 0:A], zer[:, 0:A].bitcast(F32R))
                nc.gpsimd.dma_start(kt[:, ko, 0:A], zf16[:, 0:A])
                nc.gpsimd.dma_start(
                    ht[:, ko, S + A:PADW], zer[:, 0:PADW - S - A].bitcast(F32R)
                )
                nc.gpsimd.dma_start(kt[:, ko, S + A:PADW], zf16[:, 0:PADW - S - A])
            nc.gpsimd.dma_start(wo_sb[:], rearr(wo))
            nc.gpsimd.dma_start(bh_sb[:], bh.rearrange("(hm p) -> p hm", p=P))

            # ---------------- per-stripe phase bodies ----------------
            def p1_stripe(t):  # ht = relu(W1^T @ xt + b1), 512 tokens
                for hm in range(2):
                    ps = psum.tile([P, 512], F32, tag="bank")
                    for k in range(2):
                        nc.tensor.matmul(
                            ps[:], w1_sb[:, k, hm * P:(hm + 1) * P],
                            xt_sb[:, k, t * 512:(t + 1) * 512],
                            start=(k == 0), stop=(k == 1),
                        )
                    nc.scalar.activation(
                        ht[:, hm, A + t * 512:A + (t + 1) * 512], ps[:],
                        mybir.ActivationFunctionType.Relu, bias=b1_sb[:, hm:hm + 1],
                    )

            def p23_stripe(t):  # qt = M^T ht (M = Wq Wk^T, host-folded)
                for hm in range(2):
                    psq = psum.tile([P, 512], F32, tag="bank")
                    for k in range(2):
                        nc.tensor.matmul(
                            psq[:], wq_sb[:, k, hm * P:(hm + 1) * P],
                            ht[:, k, A + t * 512:A + (t + 1) * 512],
                            start=(k == 0), stop=(k == 1),
                        )
                    nc.scalar.copy(qt[:, hm, t * 512:(t + 1) * 512], psq[:])
                # K side is h itself: fp16 cast (2x_2P SBUF-to-SBUF)
                nc.vector.tensor_copy(
                    kt[:, :, A + t * 512:A + (t + 1) * 512],
                    ht[:, :, A + t * 512:A + (t + 1) * 512],
                )

            def p4_tile(v):  # shifted V tile (natural layout, fp16)
                psv = psum.tile([P, D], F32, tag="bank")
                for k in range(2):
                    nc.tensor.matmul(
                        psv[:], ht[:, k, v * P:(v + 1) * P], wv_sb[:, k, :],
                        start=(k == 0), stop=(k == 1),
                    )
                nc.vector.tensor_copy(vs[:, v, :], psv[:])

            # ---- band attention, software-pipelined per chunk-pair ----
            # stage A (PE): mask-init + QK logits for both chunks into one bank
            # stage B (ACT/DVE): exp+rowsum, recip, normalize (fp16)
            # stage C (PE/DVE): transpose weights, apply V, copy att out
            pair_state = {}

            def p5_logits(cp):
                psl = psum.tile([P, 2 * WINW], F32, tag="bank", name="logit")
                nc.tensor.matmul(psl[:], id_h[:], mk_h[:], start=True, stop=False)
                for ci in range(2):
                    c = 2 * cp + ci
                    for k in range(2):
                        nc.tensor.matmul(
                            psl[:, ci * WINW:(ci + 1) * WINW],
                            qt[:, k, c * P:(c + 1) * P],
                            kt[:, k, c * P:c * P + WINW],
                            start=False, stop=(ci == 1 and k == 1),
                        )
                pair_state[cp] = psl

            def p5_softmax(cp):
                psl = pair_state.pop(cp)
                enb = work.tile([P, 2 * WINW], FP16, tag="enb")
                for ci in range(2):
                    sl = slice(ci * WINW, (ci + 1) * WINW)
                    e = work.tile([P, WINW], FP16, tag="e")
                    den = work.tile([P, 1], F32, tag="den")
                    nc.scalar.activation(
                        e[:], psl[:, sl], mybir.ActivationFunctionType.Exp,
                        scale=0.0625, accum_out=den[:],
                    )
                    rec = work.tile([P, 1], F32, tag="rec")
                    nc.vector.reciprocal(rec[:], den[:])
                    nc.vector.tensor_scalar_mul(enb[:, sl], e[:], rec[:])
                pair_state[("enb", cp)] = enb

            def p5_apply(cp):
                enb = pair_state.pop(("enb", cp))
                pse = psum.tile([P, 4, P], FP16, tag="bank", name="etr")
                for ci in range(2):
                    nc.tensor.transpose(
                        pse[:, 2 * ci, :],
                        enb[:, ci * WINW:ci * WINW + P], id_h[:]
                    )
                    nc.tensor.transpose(
                        pse[0:2 * A, 2 * ci + 1, :],
                        enb[:, ci * WINW + P:(ci + 1) * WINW], id_h[:]
                    )
                et = work.tile([P, 4, P], FP16, tag="et")
                nc.vector.tensor_copy(et[:], pse[:])
                psa = psum.tile([P, 2, 2 * P], F32, tag="bank", name="attp")
                for ci in range(2):
                    c = 2 * cp + ci
                    for fm in range(2):
                        nc.tensor.matmul(
                            psa[:, fm, ci * P:(ci + 1) * P],
                            vs[:, c, fm * P:(fm + 1) * P],
                            et[:, 2 * ci, :],
                            start=True, stop=False,
                        )
                        nc.tensor.matmul(
                            psa[:, fm, ci * P:(ci + 1) * P],
                            vs[0:2 * A, c + 1, fm * P:(fm + 1) * P],
                            et[0:2 * A, 2 * ci + 1, :],
                            start=False, stop=True,
                        )
                for fm in range(2):
                    nc.scalar.activation(
                        hid[:, fm, cp * 2 * P:(cp + 1) * 2 * P], psa[:, fm, :],
                        mybir.ActivationFunctionType.Relu, bias=bh_sb[:, fm:fm + 1],
                    )

            def p7_piece(u):  # out^T = Wo^T @ hid, 512 tokens + stream out
                pso = psum.tile([2, 512], F32, tag="bank", name="outp")
                for k in range(2):
                    nc.tensor.matmul(
                        pso[:], wo_sb[:, k, :],
                        hid[:, k, u * 512:(u + 1) * 512],
                        start=(k == 0), stop=(k == 1),
                    )
                nc.scalar.copy(ot_sb[:, u * 512:(u + 1) * 512], pso[:])
                nc.sync.dma_start(
                    out_t[:, u * 512:(u + 1) * 512],
                    ot_sb[:, u * 512:(u + 1) * 512],
                )

            # ---------------- token-stripe wave + pipelined attention -------
            # Stage skew keeps each engine's stream from blocking on the
            # cross-engine round trip: logits(cp) run ~2 pair-stages ahead of
            # apply(cp).
            rounds = NC // 2
            lg = sm = ap = p6u = 0

            def flush_p6():
                nonlocal p6u
                while p6u < ap // 2:
                    p7_piece(p6u)
                    p6u += 1

            for t in range(4):
                p1_stripe(t)
                p23_stripe(t)
                for v in range(4 * t, 4 * t + 4):
                    p4_tile(v)
                if t == 3:
                    p4_tile(NC)
                max_chunk = 4 * t + 2 if t < 3 else NC - 1
                max_lg = (max_chunk - 1) // 2
                max_ap = (4 * t + 1) // 2 if t < 3 else rounds - 1
                while lg <= max_lg:
                    p5_logits(lg)
                    lg += 1
                    if sm < lg - 1:
                        p5_softmax(sm)
                        sm += 1
                    if ap < sm - 1 and ap <= max_ap:
                        p5_apply(ap)
                        ap += 1
                        flush_p6()
            while sm < rounds:
                p5_softmax(sm)
                sm += 1
                while ap < sm - 1:
                    p5_apply(ap)
                    ap += 1
                    flush_p6()
            while ap < rounds:
                p5_apply(ap)
                ap += 1
                flush_p6()

    nc.compile()
    return nc


def _get_nc():
    global _CACHED_NC
    if _CACHED_NC is None:
        _CACHED_NC = _build_nc()
    return _CACHED_NC


def _band_mask():
    j = np.arange(WINW)[None, :]
    p = np.arange(P)[:, None]
    m = np.where((j >= p) & (j <= p + 2 * A), 0.0, -60000.0).astype(np.float16)
    return np.tile(m, (1, 2))


def kernel(x, W1, b1, Wq, Wk, Wv, Wh, bh, Wo, bo, **_unused):
    x = np.asarray(x, dtype=np.float32)
    W1 = np.asarray(W1, dtype=np.float32)
    Wq = np.asarray(Wq, dtype=np.float32)
    Wk = np.asarray(Wk, dtype=np.float32)
    Wv = np.asarray(Wv, dtype=np.float32)
    Wh = np.asarray(Wh, dtype=np.float32)
    Wo = np.asarray(Wo, dtype=np.float32)
    b1f = np.asarray(b1, dtype=np.float32).reshape(D)
    bhf = np.asarray(bh, dtype=np.float32).reshape(D)
    bof = np.asarray(bo, dtype=np.float32).reshape(2)
    zer = np.zeros((P, 2 * P), dtype=np.float32)
    zerh = np.zeros((P, 2 * P), dtype=np.float16)
    idh = np.eye(P, dtype=np.float16)
    mkb = _band_mask()

    wqm = (Wq.astype(np.float64) @ Wk.astype(np.float64).T).astype(np.float32)
    wvh = (Wv.astype(np.float64) @ Wh.astype(np.float64)).astype(np.float32)

    nc = _get_nc()
    in_maps = []
    for b in range(NCORES):
        in_maps.append({
            "xt": np.ascontiguousarray(x[b].T),
            "w1": W1, "wq": wqm, "wv": wvh, "wo": Wo,
            "b1": b1f, "bh": bhf, "zer": zer, "zerh": zerh,
            "idh": idh, "mkb": mkb,
        })
    res = run_bass_kernel_spmd(nc, in_maps, core_ids=list(range(NCORES)))
    global _LAST_RESULTS
    _LAST_RESULTS = res
    out = np.stack(
        [res.results[b]["out_t"].T + bof[None, :] for b in range(NCORES)], axis=0
    )
    return out.astype(np.float32)


if __name__ == "__main__":
    rng = np.random.default_rng(0)
    ins = {
        "x": rng.standard_normal((8, S, D), dtype=np.float32),
        "W1": (rng.standard_normal((D, D), dtype=np.float32) / 16),
        "b1": np.zeros((1, 1, D), np.float32),
        "Wq": (rng.standard_normal((D, D), dtype=np.float32) / 16),
        "Wk": (rng.standard_normal((D, D), dtype=np.float32) / 16),
        "Wv": (rng.standard_normal((D, D), dtype=np.float32) / 16),
        "Wh": (rng.standard_normal((D, D), dtype=np.float32) / 16),
        "bh": np.zeros((1, 1, D), np.float32),
        "Wo": (rng.standard_normal((D, 2), dtype=np.float32) / 16),
        "bo": np.zeros((1, 1, 2), np.float32),
    }
    y = kernel(**ins)
    print("kernel output", y.shape, y.dtype, float(np.abs(y).max()))


# revision 29
# speedup vs baseline: 1.8603x; 1.0319x over previous
"""Trainium2 Bass kernel for windowed local self-attention MLP.

Reference computation (per batch b, S=2048 tokens, D=H=256, A=16, W=33):
    h   = relu(x @ W1 + b1)
    Q   = h @ Wq ; Khat = h @ Wk ; Vhat = h @ Wv        (windowed K/V are
          shifted views of Khat/Vhat -- the algebraic collapse of the
          reference's [B,S,W,H] window tensor)
    logit[s,m] = Q[s].Khat[s+A-m]/sqrt(H)  (zero outside [0,S), m=0..32)
    attn = softmax(logit) ; att[s] = sum_m attn[s,m] Vhat[s+A-m]
    out = relu(att @ Wh + bh) @ Wo + bo

Sharding: data-parallel over batch, one batch element per NeuronCore (B=8,
8 cores), weights replicated, no collectives.

Layout: activations feature-on-partition ([256=2x128, S]) so dense matmuls
contract over partitions; x is transposed host-side. Band attention runs per
128-token chunk over a 256-token window aligned to shifted (-A) 128-tiles.
The additive -1e9 band mask is pre-loaded into PSUM by an identity matmul,
the QK logits accumulate on top, exp runs with a fused row-sum (accum_out),
and a PE transpose of the normalized fp16 weights feeds [V-tile]^T @ [e]^T
fp16 matmuls. Dense matmuls use float32r (full PE rate, ~2e-4 rel err).
Emission is a token-stripe wave so DMA/PE/ACT/DVE overlap across phases.
Final projection is emitted transposed [2, S]; un-transposed and bo added
host-side.
"""
import sys

if "/opt/trn_rl_repo" not in sys.path:
    sys.path.insert(0, "/opt/trn_rl_repo")

import numpy as np

import concourse.mybir as mybir
import concourse.tile as tile
from concourse import bacc
from concourse.bass_utils import run_bass_kernel_spmd

P = 128
S = 2048  # tokens per core
D = 256  # model dim
A = 16  # half window
NC = 16  # token chunks per core
NCORES = 8

PADW = P * (NC + 1)  # 2176: padded token axis, col = token + A
WINW = P + 2 * A  # 160: per-chunk attention window
F32 = mybir.dt.float32
F32R = mybir.dt.float32r
FP16 = mybir.dt.float16

_CACHED_NC = None
_LAST_RESULTS = None


def _build_nc():
    nc = bacc.Bacc(
        "TRN2",
        target_bir_lowering=False,
        debug=False,
        enable_asserts=False,
        num_devices=NCORES,
    )
    xt = nc.dram_tensor("xt", [D, S], F32, kind="ExternalInput").ap()
    w1 = nc.dram_tensor("w1", [D, D], F32, kind="ExternalInput").ap()
    wq = nc.dram_tensor("wq", [D, D], F32, kind="ExternalInput").ap()
    wv = nc.dram_tensor("wv", [D, D], F32, kind="ExternalInput").ap()
    wo = nc.dram_tensor("wo", [D, 2], F32, kind="ExternalInput").ap()
    b1 = nc.dram_tensor("b1", [D], F32, kind="ExternalInput").ap()
    bh = nc.dram_tensor("bh", [D], F32, kind="ExternalInput").ap()
    idh = nc.dram_tensor("idh", [P, P], FP16, kind="ExternalInput").ap()
    mkb = nc.dram_tensor("mkb", [P, 2 * WINW], FP16, kind="ExternalInput").ap()
    zer = nc.dram_tensor("zer", [P, 2 * P], F32, kind="ExternalInput").ap()
    zerh = nc.dram_tensor("zerh", [P, 2 * P], FP16, kind="ExternalInput").ap()
    out_t = nc.dram_tensor("out_t", [2, S], F32, kind="ExternalOutput").ap()

    with tile.TileContext(nc) as tc:
        with (
            tc.tile_pool(name="persist", bufs=1) as persist,
            tc.tile_pool(name="work", bufs=6) as work,
            tc.tile_pool(name="psum", bufs=8, space="PSUM") as psum,
        ):
            # ---------------- persistent tiles ----------------
            w1_sb = persist.tile([P, 2, D], F32R)
            wq_sb = persist.tile([P, 2, D], F32R)
            wv_sb = persist.tile([P, 2, D], F32R)
            wo_sb = persist.tile([P, 2, 2], F32R)
            b1_sb = persist.tile([P, 2], F32)
            bh_sb = persist.tile([P, 2], F32)
            id_h = persist.tile([P, P], FP16)
            mk_h = persist.tile([P, 2 * WINW], FP16)

            xt_sb = persist.tile([P, 2, S], F32R)
            zf16 = zerh
            ht = persist.tile([P, 2, PADW], F32R)  # col = token + A
            qt = persist.tile([P, 2, S], FP16)
            kt = persist.tile([P, 2, PADW], FP16)  # col = token + A
            vs = persist.tile([P, NC + 1, D], FP16)  # tile t row p = token t*128+p-A
            hid = persist.tile([P, 2, S], F32R)
            ot_sb = persist.tile([2, S], F32)

            def rearr(w):
                return w.rearrange("(k p) h -> p k h", p=P).bitcast(F32R)

            # ---- startup DMAs, split across both HWDGE rings ----
            xtr = xt.rearrange("(ko p) s -> p ko s", p=P).bitcast(F32R)
            nc.sync.dma_start(xt_sb[:, 0, 0:512], xtr[:, 0, 0:512])
            nc.scalar.dma_start(w1_sb[:], rearr(w1))
            nc.sync.dma_start(xt_sb[:, 1, 0:512], xtr[:, 1, 0:512])
            nc.scalar.dma_start(b1_sb[:], b1.rearrange("(hm p) -> p hm", p=P))
            nc.scalar.dma_start(wq_sb[:], rearr(wq))
            nc.sync.dma_start(xt_sb[:, 0, 512:S], xtr[:, 0, 512:S])
            nc.scalar.dma_start(xt_sb[:, 1, 512:S], xtr[:, 1, 512:S])
            # non-critical loads ride SWDGE on the otherwise-idle Pool engine
            nc.gpsimd.dma_start(wv_sb[:], rearr(wv))
            nc.gpsimd.dma_start(id_h[:], idh)
            nc.gpsimd.dma_start(mk_h[:], mkb)
            for ko in range(2):
                nc.gpsimd.dma_start(ht[:, ko, 0:A], zer[:, 0:A].bitcast(F32R))
                nc.gpsimd.dma_start(kt[:, ko, 0:A], zf16[:, 0:A])
                nc.gpsimd.dma_start(
                    ht[:, ko, S + A:PADW], zer[:, 0:PADW - S - A].bitcast(F32R)
                )
                nc.gpsimd.dma_start(kt[:, ko, S + A:PADW], zf16[:, 0:PADW - S - A])
            nc.gpsimd.dma_start(wo_sb[:], rearr(wo))
            nc.gpsimd.dma_start(bh_sb[:], bh.rearrange("(hm p) -> p hm", p=P))

            # ---------------- per-stripe phase bodies ----------------
            def p1_stripe(t):  # ht = relu(W1^T @ xt + b1), 512 tokens
                for hm in range(2):
                    ps = psum.tile([P, 512], F32, tag="bank")
                    for k in range(2):
                        nc.tensor.matmul(
                            ps[:], w1_sb[:, k, hm * P:(hm + 1) * P],
                            xt_sb[:, k, t * 512:(t + 1) * 512],
                            start=(k == 0), stop=(k == 1),
                        )
                    if hm == 0:
                        nc.scalar.activation(
                            ht[:, hm, A + t * 512:A + (t + 1) * 512], ps[:],
                            mybir.ActivationFunctionType.Relu,
                            bias=b1_sb[:, hm:hm + 1],
                        )
                    else:
                        nc.vector.tensor_scalar(
                            ht[:, hm, A + t * 512:A + (t + 1) * 512], ps[:],
                            b1_sb[:, hm:hm + 1], 0.0,
                            mybir.AluOpType.add, mybir.AluOpType.max,
                        )

            def p23_stripe(t):  # qt = M^T ht (M = Wq Wk^T, host-folded)
                for hm in range(2):
                    psq = psum.tile([P, 512], F32, tag="bank")
                    for k in range(2):
                        nc.tensor.matmul(
                            psq[:], wq_sb[:, k, hm * P:(hm + 1) * P],
                            ht[:, k, A + t * 512:A + (t + 1) * 512],
                            start=(k == 0), stop=(k == 1),
                        )
                    nc.scalar.copy(qt[:, hm, t * 512:(t + 1) * 512], psq[:])
                # K side is h itself: fp16 cast (2x_2P SBUF-to-SBUF)
                nc.vector.tensor_copy(
                    kt[:, :, A + t * 512:A + (t + 1) * 512],
                    ht[:, :, A + t * 512:A + (t + 1) * 512],
                )

            def p4_tile(v):  # shifted V tile (natural layout, fp16)
                psv = psum.tile([P, D], F32, tag="bank")
                for k in range(2):
                    nc.tensor.matmul(
                        psv[:], ht[:, k, v * P:(v + 1) * P], wv_sb[:, k, :],
                        start=(k == 0), stop=(k == 1),
                    )
                nc.vector.tensor_copy(vs[:, v, :], psv[:])

            # ---- band attention, software-pipelined per chunk-pair ----
            # stage A (PE): mask-init + QK logits for both chunks into one bank
            # stage B (ACT/DVE): exp+rowsum, recip, normalize (fp16)
            # stage C (PE/DVE): transpose weights, apply V, copy att out
            pair_state = {}

            def p5_logits(cp):
                psl = psum.tile([P, 2 * WINW], F32, tag="bank", name="logit")
                nc.tensor.matmul(psl[:], id_h[:], mk_h[:], start=True, stop=False)
                for ci in range(2):
                    c = 2 * cp + ci
                    for k in range(2):
                        nc.tensor.matmul(
                            psl[:, ci * WINW:(ci + 1) * WINW],
                            qt[:, k, c * P:(c + 1) * P],
                            kt[:, k, c * P:c * P + WINW],
                            start=False, stop=(ci == 1 and k == 1),
                        )
                pair_state[cp] = psl

            def p5_softmax(cp):
                psl = pair_state.pop(cp)
                enb = work.tile([P, 2 * WINW], FP16, tag="enb")
                for ci in range(2):
                    sl = slice(ci * WINW, (ci + 1) * WINW)
                    e = work.tile([P, WINW], FP16, tag="e")
                    den = work.tile([P, 1], F32, tag="den")
                    nc.scalar.activation(
                        e[:], psl[:, sl], mybir.ActivationFunctionType.Exp,
                        scale=0.0625, accum_out=den[:],
                    )
                    rec = work.tile([P, 1], F32, tag="rec")
                    nc.vector.reciprocal(rec[:], den[:])
                    nc.vector.tensor_scalar_mul(enb[:, sl], e[:], rec[:])
                pair_state[("enb", cp)] = enb

            def p5_apply(cp):
                enb = pair_state.pop(("enb", cp))
                pse = psum.tile([P, 4, P], FP16, tag="bank", name="etr")
                for ci in range(2):
                    nc.tensor.transpose(
                        pse[:, 2 * ci, :],
                        enb[:, ci * WINW:ci * WINW + P], id_h[:]
                    )
                    nc.tensor.transpose(
                        pse[0:2 * A, 2 * ci + 1, :],
                        enb[:, ci * WINW + P:(ci + 1) * WINW], id_h[:]
                    )
                et = work.tile([P, 4, P], FP16, tag="et")
                nc.vector.tensor_copy(et[:], pse[:])
                psa = psum.tile([P, 2, 2 * P], F32, tag="bank", name="attp")
                for ci in range(2):
                    c = 2 * cp + ci
                    for fm in range(2):
                        nc.tensor.matmul(
                            psa[:, fm, ci * P:(ci + 1) * P],
                            vs[:, c, fm * P:(fm + 1) * P],
                            et[:, 2 * ci, :],
                            start=True, stop=False,
                        )
                        nc.tensor.matmul(
                            psa[:, fm, ci * P:(ci + 1) * P],
                            vs[0:2 * A, c + 1, fm * P:(fm + 1) * P],
                            et[0:2 * A, 2 * ci + 1, :],
                            start=False, stop=True,
                        )
                nc.scalar.activation(
                    hid[:, 0, cp * 2 * P:(cp + 1) * 2 * P], psa[:, 0, :],
                    mybir.ActivationFunctionType.Relu, bias=bh_sb[:, 0:1],
                )
                nc.vector.tensor_scalar(
                    hid[:, 1, cp * 2 * P:(cp + 1) * 2 * P], psa[:, 1, :],
                    bh_sb[:, 1:2], 0.0,
                    mybir.AluOpType.add, mybir.AluOpType.max,
                )

            def p7_piece(u):  # out^T = Wo^T @ hid, 512 tokens + stream out
                pso = psum.tile([2, 512], F32, tag="bank", name="outp")
                for k in range(2):
                    nc.tensor.matmul(
                        pso[:], wo_sb[:, k, :],
                        hid[:, k, u * 512:(u + 1) * 512],
                        start=(k == 0), stop=(k == 1),
                    )
                nc.scalar.copy(ot_sb[:, u * 512:(u + 1) * 512], pso[:])
                nc.sync.dma_start(
                    out_t[:, u * 512:(u + 1) * 512],
                    ot_sb[:, u * 512:(u + 1) * 512],
                )

            # ---------------- token-stripe wave + pipelined attention -------
            # Stage skew keeps each engine's stream from blocking on the
            # cross-engine round trip: logits(cp) run ~2 pair-stages ahead of
            # apply(cp).
            rounds = NC // 2
            lg = sm = ap = p6u = 0

            def flush_p6():
                nonlocal p6u
                while p6u < ap // 2:
                    p7_piece(p6u)
                    p6u += 1

            for t in range(4):
                p1_stripe(t)
                p23_stripe(t)
                for v in range(4 * t, 4 * t + 4):
                    p4_tile(v)
                if t == 3:
                    p4_tile(NC)
                max_chunk = 4 * t + 2 if t < 3 else NC - 1
                max_lg = (max_chunk - 1) // 2
                max_ap = (4 * t + 1) // 2 if t < 3 else rounds - 1
                while lg <= max_lg:
                    p5_logits(lg)
                    lg += 1
                    if sm < lg - 1:
                        p5_softmax(sm)
                        sm += 1
                    if ap < sm - 1 and ap <= max_ap:
                        p5_apply(ap)
                        ap += 1
                        flush_p6()
            while sm < rounds:
                p5_softmax(sm)
                sm += 1
                while ap < sm - 1:
                    p5_apply(ap)
                    ap += 1
                    flush_p6()
            while ap < rounds:
                p5_apply(ap)
                ap += 1
                flush_p6()

    nc.compile()
    return nc


def _get_nc():
    global _CACHED_NC
    if _CACHED_NC is None:
        _CACHED_NC = _build_nc()
    return _CACHED_NC


def _band_mask():
    j = np.arange(WINW)[None, :]
    p = np.arange(P)[:, None]
    m = np.where((j >= p) & (j <= p + 2 * A), 0.0, -60000.0).astype(np.float16)
    return np.tile(m, (1, 2))


def kernel(x, W1, b1, Wq, Wk, Wv, Wh, bh, Wo, bo, **_unused):
    x = np.asarray(x, dtype=np.float32)
    W1 = np.asarray(W1, dtype=np.float32)
    Wq = np.asarray(Wq, dtype=np.float32)
    Wk = np.asarray(Wk, dtype=np.float32)
    Wv = np.asarray(Wv, dtype=np.float32)
    Wh = np.asarray(Wh, dtype=np.float32)
    Wo = np.asarray(Wo, dtype=np.float32)
    b1f = np.asarray(b1, dtype=np.float32).reshape(D)
    bhf = np.asarray(bh, dtype=np.float32).reshape(D)
    bof = np.asarray(bo, dtype=np.float32).reshape(2)
    zer = np.zeros((P, 2 * P), dtype=np.float32)
    zerh = np.zeros((P, 2 * P), dtype=np.float16)
    idh = np.eye(P, dtype=np.float16)
    mkb = _band_mask()

    wqm = (Wq.astype(np.float64) @ Wk.astype(np.float64).T).astype(np.float32)
    wvh = (Wv.astype(np.float64) @ Wh.astype(np.float64)).astype(np.float32)

    nc = _get_nc()
    in_maps = []
    for b in range(NCORES):
        in_maps.append({
            "xt": np.ascontiguousarray(x[b].T),
            "w1": W1, "wq": wqm, "wv": wvh, "wo": Wo,
            "b1": b1f, "bh": bhf, "zer": zer, "zerh": zerh,
            "idh": idh, "mkb": mkb,
        })
    res = run_bass_kernel_spmd(nc, in_maps, core_ids=list(range(NCORES)))
    global _LAST_RESULTS
    _LAST_RESULTS = res
    out = np.stack(
        [res.results[b]["out_t"].T + bof[None, :] for b in range(NCORES)], axis=0
    )
    return out.astype(np.float32)


if __name__ == "__main__":
    rng = np.random.default_rng(0)
    ins = {
        "x": rng.standard_normal((8, S, D), dtype=np.float32),
        "W1": (rng.standard_normal((D, D), dtype=np.float32) / 16),
        "b1": np.zeros((1, 1, D), np.float32),
        "Wq": (rng.standard_normal((D, D), dtype=np.float32) / 16),
        "Wk": (rng.standard_normal((D, D), dtype=np.float32) / 16),
        "Wv": (rng.standard_normal((D, D), dtype=np.float32) / 16),
        "Wh": (rng.standard_normal((D, D), dtype=np.float32) / 16),
        "bh": np.zeros((1, 1, D), np.float32),
        "Wo": (rng.standard_normal((D, 2), dtype=np.float32) / 16),
        "bo": np.zeros((1, 1, 2), np.float32),
    }
    y = kernel(**ins)
    print("kernel output", y.shape, y.dtype, float(np.abs(y).max()))
